# revision 23
# baseline (speedup 1.0000x reference)
"""Trainium2 Bass kernel for nn_Block_9457517985872 (dense transformer block
with linear attention). Token-sharded across 8 NeuronCores: core c handles
batch c//2, sequence half c%2 (2048 tokens). Only cross-core communication is
a pairwise AllReduce of the per-head (kv, ksum) statistics [16,64,65] f32.

Self-contained: hardcodes all shapes from the problem spec.
"""
import numpy as np
from contextlib import ExitStack

import concourse.bass as bass
import concourse.tile as tile
from concourse import bacc, mybir
from concourse.bass_utils import run_bass_kernel_spmd
from concourse.masks import make_identity

F32 = mybir.dt.float32
F32R = mybir.dt.float32r
AF = mybir.ActivationFunctionType
ALU = mybir.AluOpType

B, N, C = 4, 4096, 1024
H, D = 16, 64
HID = 4096
TOK = 2048          # tokens per core
NT = TOK // 128     # 16 token tiles
NG = TOK // 512     # 4 token groups
EPS_LN = 1e-5
EPS_ATTN = 1e-6

_BUILD_CACHE = {}


def _emit_ln(nc, pools, x_t, eps_t, out_t):
    """LayerNorm core (no gamma/beta): out = (x - mean(x)) * rsqrt(var + eps).
    x_t: [128, 1024] f32 SBUF tile. out_t may alias x_t."""
    stats = pools["stat"].tile([128, 2, 6], F32, name="ln_stats", tag="ln_stats")
    mv = pools["stat"].tile([128, 2], F32, name="ln_mv", tag="ln_mv")
    for sg in range(2):
        nc.vector.bn_stats(out=stats[:, sg, :], in_=x_t[:, sg * 512:(sg + 1) * 512])
    nc.vector.bn_aggr(out=mv[:], in_=stats[:])
    # mv[:,0]=mean, mv[:,1]=var -> rstd
    nc.scalar.activation(out=mv[:, 1:2], in_=mv[:, 1:2], func=AF.Sqrt, bias=eps_t[:], scale=1.0)
    nc.vector.reciprocal(out=mv[:, 1:2], in_=mv[:, 1:2])
    # mv[:,0] = -mean*rstd
    nc.vector.tensor_tensor(out=mv[:, 0:1], in0=mv[:, 0:1], in1=mv[:, 1:2], op=ALU.mult)
    nc.vector.tensor_scalar_mul(out=mv[:, 0:1], in0=mv[:, 0:1], scalar1=-1.0)
    nc.scalar.activation(out=out_t[:], in_=x_t[:], func=AF.Identity,
                         bias=mv[:, 0:1], scale=mv[:, 1:2])


def _build(flags, no_cc=False):
    """flags: (has_bk, has_bv, has_bproj, has_bfc2)"""
    has_bk, has_bv, has_bproj, has_bfc2 = flags
    nc = bacc.Bacc("TRN2", target_bir_lowering=False, debug=False,
                   num_devices=1 if no_cc else 8)

    xs = nc.dram_tensor("xs", [TOK, C], F32, kind="ExternalInput")
    wq = nc.dram_tensor("wq", [C, C], F32, kind="ExternalInput")      # [c, o]
    wkv = nc.dram_tensor("wkv", [C, 2 * C], F32, kind="ExternalInput")
    wp = nc.dram_tensor("wp", [C, C], F32, kind="ExternalInput")
    w1 = nc.dram_tensor("w1", [C, HID], F32, kind="ExternalInput")
    w2 = nc.dram_tensor("w2", [HID, C], F32, kind="ExternalInput")
    bq = nc.dram_tensor("bq", [C], F32, kind="ExternalInput")
    bk = nc.dram_tensor("bk", [C], F32, kind="ExternalInput")
    bv = nc.dram_tensor("bv", [C], F32, kind="ExternalInput")
    bg = nc.dram_tensor("bg", [HID], F32, kind="ExternalInput")
    bp = nc.dram_tensor("bp", [C], F32, kind="ExternalInput")
    b2o = nc.dram_tensor("b2o", [C], F32, kind="ExternalInput")
    out = nc.dram_tensor("out", [TOK, C], F32, kind="ExternalOutput")

    xs_v = xs.ap().rearrange("(t p) c -> t p c", p=128)     # [16,128,1024]
    out_v = out.ap().rearrange("(t p) c -> t p c", p=128)

    with tile.TileContext(nc) as tc, ExitStack() as ctx:
        const = ctx.enter_context(tc.tile_pool(name="const", bufs=1))
        dram = ctx.enter_context(tc.tile_pool(name="dram", bufs=1, space="DRAM"))
        statp = ctx.enter_context(tc.tile_pool(name="stat", bufs=4))
        pools = {"stat": statp}

        ident = const.tile([128, 128], F32)
        make_identity(nc, ident[:])
        eps_ln_t = const.tile([128, 1], F32)
        nc.vector.memset(eps_ln_t[:], EPS_LN)
        bq_sb = const.tile([128, 8], F32)
        nc.sync.dma_start(out=bq_sb[:], in_=bq.ap().rearrange("(oc p) -> p oc", p=128))
        bg_sb = const.tile([128, 32], F32)
        nc.sync.dma_start(out=bg_sb[:], in_=bg.ap().rearrange("(hd p) -> p hd", p=128))
        if has_bk:
            bk_bc = const.tile([128, C], F32)
            nc.sync.dma_start(out=bk_bc[:], in_=bass.AP(
                tensor=bk.ap().tensor, offset=0, ap=[[0, 128], [1, C]]))
        if has_bproj:
            bp_bc = const.tile([128, C], F32)
            nc.sync.dma_start(out=bp_bc[:], in_=bass.AP(
                tensor=bp.ap().tensor, offset=0, ap=[[0, 128], [1, C]]))
        if has_bfc2:
            b2_bc = const.tile([128, C], F32)
            nc.sync.dma_start(out=b2_bc[:], in_=bass.AP(
                tensor=b2o.ap().tensor, offset=0, ap=[[0, 128], [1, C]]))

        x1s = dram.tile([NT, 128, C], F32)
        h3s = dram.tile([32, 128, TOK], F32)
        cci = dram.tile([2, 128, 4, 65], F32)
        cco = dram.tile([2, 128, 4, 65], F32)
        z_d = dram.tile([16, TOK], F32)
        ht_d = dram.tile([NT, 128, 8, 128], F32)

        # ---------------- Phase 1a: LN1, hT, k/v, kv+ksum ----------------
        with (
            tc.tile_pool(name="wkvp", bufs=1) as wkvp,
            tc.tile_pool(name="p1w", bufs=3) as p1w,
            tc.tile_pool(name="htrp", bufs=2) as htrp,
            tc.tile_pool(name="trtmp", bufs=2) as trtmpp,
            tc.tile_pool(name="kvstage", bufs=1) as kvstagep,
            tc.tile_pool(name="kvacc_ps", bufs=1, space="PSUM") as kvaccp,
            tc.tile_pool(name="tr_ps", bufs=1, space="PSUM") as trpsp,
            tc.tile_pool(name="gen_ps", bufs=3, space="PSUM") as genpsp,
        ):
            wkv_sb = wkvp.tile([128, 8, 2 * C], F32R)
            wkv_v = wkv.ap().rearrange("(cc p) o -> p cc o", p=128).bitcast(F32R)
            for cc in range(8):
                nc.sync.dma_start(out=wkv_sb[:, cc, :], in_=wkv_v[:, cc, :])
            kv_ps = [kvaccp.tile([128, 4, 65], F32, name=f"kv_ps{i}") for i in range(2)]

            for tt in range(NT):
                x_t = p1w.tile([128, C], F32, tag="x")
                nc.sync.dma_start(out=x_t[:], in_=xs_v[tt])
                _emit_ln(nc, pools, x_t, eps_ln_t, x_t)
                # transpose h (=x_t) -> hT_full[:, :, tt*128:+128]
                tr_ps = trpsp.tile([128, 8, 128], F32)
                for cc in range(8):
                    nc.tensor.transpose(tr_ps[:, cc, :], x_t[:, cc * 128:(cc + 1) * 128], ident[:])
                tr_tmp = trtmpp.tile([128, 8, 128], F32)
                nc.vector.tensor_copy(out=tr_tmp[:], in_=tr_ps[:])
                nc.sync.dma_start(out=ht_d[tt], in_=tr_tmp[:])
                hT_r = htrp.tile([128, 8, 128], F32R)
                nc.sync.dma_start(out=hT_r[:], in_=tr_tmp[:].bitcast(F32R))
                # k, v for this tile
                k_sb = p1w.tile([128, C], F32, tag="k")
                v_ext = p1w.tile([128, H, 65], F32, tag="v")
                nc.vector.memset(v_ext[:, :, 64:65], 1.0)
                for oc in range(4):
                    ps = genpsp.tile([128, 512], F32, tag="gen")
                    for cc in range(8):
                        nc.tensor.matmul(ps[:], lhsT=hT_r[:, cc, :],
                                         rhs=wkv_sb[:, cc, oc * 512:(oc + 1) * 512],
                                         start=(cc == 0), stop=(cc == 7))
                    if oc < 2:  # k: phi = exp(min(x,0)) + relu(x)
                        ksl = k_sb[:, oc * 512:(oc + 1) * 512]
                        if has_bk:
                            nc.vector.tensor_tensor(out=ksl, in0=ps[:],
                                                    in1=bk_bc[:, oc * 512:(oc + 1) * 512], op=ALU.add)
                            src = ksl
                        else:
                            src = ps[:]
                        mt = p1w.tile([128, 512], F32, tag="phim")
                        nc.vector.tensor_scalar_min(out=mt[:], in0=src, scalar1=0.0)
                        nc.scalar.activation(out=mt[:], in_=mt[:], func=AF.Exp)
                        nc.vector.scalar_tensor_tensor(out=ksl, in0=src, scalar=0.0,
                                                       in1=mt[:], op0=ALU.max, op1=ALU.add)
                    else:      # v -> v_ext[:, heads, 0:64]
                        h0 = (oc - 2) * 8
                        dst = v_ext[:, h0:h0 + 8, 0:64]
                        if has_bv:
                            vb = bass.AP(tensor=bv.ap().tensor, offset=(oc - 2) * 512,
                                         ap=[[0, 128], [64, 8], [1, 64]])
                            vb_t = p1w.tile([128, 8, 64], F32, tag="vb")
                            nc.sync.dma_start(out=vb_t[:], in_=vb)
                            nc.vector.tensor_tensor(
                                out=dst, in0=ps[:].rearrange("p (h d) -> p h d", d=64),
                                in1=vb_t[:], op=ALU.add)
                        else:
                            nc.vector.tensor_copy(
                                out=dst, in_=ps[:].rearrange("p (h d) -> p h d", d=64))
                # kv accumulation: per head [64, 65] += k_h^T @ [v_h | 1]
                for h in range(H):
                    ti, hf, slot = h // 8, (h % 8) // 4, h % 4
                    nc.tensor.matmul(
                        kv_ps[ti][hf * 64:(hf + 1) * 64, slot, :],
                        lhsT=k_sb[:, h * 64:(h + 1) * 64],
                        rhs=v_ext[:, h, :],
                        start=(tt == 0), stop=(tt == NT - 1))

            # stage kv psum -> SBUF -> DRAM, then pairwise AllReduce
            kv_st = kvstagep.tile([128, 2, 4, 65], F32)
            for ti in range(2):
                nc.vector.tensor_copy(out=kv_st[:, ti], in_=kv_ps[ti][:])
                nc.sync.dma_start(out=cci[ti], in_=kv_st[:, ti])
            if no_cc:
                nc.sync.dma_start(out=cco[:], in_=cci[:])
            else:
                nc.gpsimd.collective_compute(
                    "AllReduce", ALU.add,
                    replica_groups=[[0, 1], [2, 3], [4, 5], [6, 7]],
                    ins=[cci[:]], outs=[cco[:]])

        # ------------- Phase 1b: qT (overlaps the collective) -------------
        big_cm = tc.tile_pool(name="big", bufs=4, side="right")
        big = big_cm.__enter__()
        qT_g = [big.tile([128, 8, 512], F32R, tag="grp", name=f"qT_g{i}") for i in range(NG)]
        with tc.tile_pool(name="wqp", bufs=2) as wqp, \
             tc.tile_pool(name="p1bw", bufs=3) as p1bw, \
             tc.tile_pool(name="qhtp", bufs=2) as qhtp, \
             tc.tile_pool(name="q_ps", bufs=4, space="PSUM") as qpsp:
            for g in range(NG):
                qht = qhtp.tile([128, 8, 4, 128], F32R)
                nc.sync.dma_start(out=qht[:], in_=ht_d[4 * g:4 * (g + 1)].rearrange(
                    "tl p cc t -> p cc tl t").bitcast(F32R))
                qht_v = qht[:].rearrange("p cc tl t -> p cc (tl t)")
                for oc in range(8):
                    wq_col = wqp.tile([128, 8, 128], F32R)
                    nc.sync.dma_start(out=wq_col[:], in_=wq.ap().rearrange(
                        "(cc p) o -> p cc o", p=128)[:, :, oc * 128:(oc + 1) * 128].bitcast(F32R))
                    ps = qpsp.tile([128, 512], F32)
                    for cc in range(8):
                        nc.tensor.matmul(ps[:], lhsT=wq_col[:, cc, :],
                                         rhs=qht_v[:, cc, :],
                                         start=(cc == 0), stop=(cc == 7))
                    mt = p1bw.tile([128, 512], F32, tag="phim")
                    rt = p1bw.tile([128, 512], F32, tag="phir")
                    nc.vector.tensor_scalar(out=mt[:], in0=ps[:], scalar1=bq_sb[:, oc:oc + 1],
                                            scalar2=0.0, op0=ALU.add, op1=ALU.min)
                    nc.scalar.activation(out=mt[:], in_=mt[:], func=AF.Exp)
                    nc.vector.tensor_scalar(out=rt[:], in0=ps[:], scalar1=bq_sb[:, oc:oc + 1],
                                            scalar2=0.0, op0=ALU.add, op1=ALU.max)
                    nc.vector.tensor_tensor(out=mt[:], in0=mt[:], in1=rt[:], op=ALU.add)
                    nc.sync.dma_start(out=qT_g[g][:, oc, :], in_=mt[:].bitcast(F32R))

        # ---------------- Phase 2: attention + proj + LN2 ----------------
        with (
            tc.tile_pool(name="wpp", bufs=1) as wpp,
            tc.tile_pool(name="kv2", bufs=1) as kv2p,
            tc.tile_pool(name="p2w", bufs=2) as p2w,
            tc.tile_pool(name="p2w1", bufs=2) as p2w1,
            tc.tile_pool(name="attnt", bufs=1) as attntp,
            tc.tile_pool(name="zbcpa", bufs=1) as zbcpa,
            tc.tile_pool(name="zbcpb", bufs=1) as zbcpb,
            tc.tile_pool(name="z_ps", bufs=2, space="PSUM") as zpsp,
            tc.tile_pool(name="attn_ps", bufs=2, space="PSUM") as attnpsp,
            tc.tile_pool(name="proj_ps", bufs=2, space="PSUM") as projpsp,
            tc.tile_pool(name="tr2_ps", bufs=1, space="PSUM") as trps2p,
        ):
            wp_sb = wpp.tile([128, 8, C], F32R)
            nc.sync.dma_start(out=wp_sb[:], in_=wp.ap().rearrange(
                "(cc p) o -> p cc o", p=128).bitcast(F32R))
            kv_sb2 = kv2p.tile([128, 8, 65], F32R)
            kv_bd = kv2p.tile([128, 8, 128], F32R)
            nc.vector.memset(kv_bd[:].bitcast(F32), 0.0)
            bd = kv2p.tile([128, 8, 16], F32R)
            nc.vector.memset(bd[:].bitcast(F32), 0.0)
            for h in range(H):
                ti, hf, slot = h // 8, (h % 8) // 4, h % 4
                pbase = (h % 2) * 64
                nc.sync.dma_start(
                    out=kv_sb2[pbase:pbase + 64, h // 2, :],
                    in_=cco[ti, hf * 64:(hf + 1) * 64, slot, :].bitcast(F32R))
                # block-diagonal kv per head pair: head h occupies rows/cols
                # [pbase, pbase+64) of kv_bd[:, h//2, :]
                nc.sync.dma_start(
                    out=kv_bd[pbase:pbase + 64, h // 2, pbase:pbase + 64],
                    in_=kv_sb2[pbase:pbase + 64, h // 2, 0:64])
                nc.sync.dma_start(
                    out=bd[pbase:pbase + 64, h // 2, h:h + 1],
                    in_=kv_sb2[pbase:pbase + 64, h // 2, 64:65])
            z_bcs = {}

            def emit_z(g):
                # z = 1 / (q . ksum + eps), then broadcast to head-pair layout
                zps = zpsp.tile([16, 512], F32, name=f"zps{g}", tag="zps")
                for pc in range(8):
                    nc.tensor.matmul(zps[:], lhsT=bd[:, pc, :], rhs=qT_g[g][:, pc, :],
                                     start=(pc == 0), stop=(pc == 7))
                zsl = p2w.tile([16, 512], F32, name=f"zt{g}", tag="zt")
                nc.vector.tensor_scalar_add(out=zsl[:], in0=zps[:], scalar1=EPS_ATTN)
                nc.vector.reciprocal(out=zsl[:], in_=zsl[:])
                nc.sync.dma_start(out=z_d[:, g * 512:(g + 1) * 512], in_=zsl[:])
                z_bc = (zbcpa if g % 2 == 0 else zbcpb).tile(
                    [128, 8, 512], F32, name=f"zbc{g}", tag="zbc")
                zd_ap = z_d[:]
                for sub in range(2):
                    nc.sync.dma_start(
                        out=z_bc[sub * 64:(sub + 1) * 64, :, :],
                        in_=bass.AP(tensor=zd_ap.tensor,
                                    offset=zd_ap.offset + sub * TOK + g * 512,
                                    ap=[[0, 64], [2 * TOK, 8], [1, 512]]))
                z_bcs[g] = z_bc

            emit_z(0)
            for g in range(NG):
                if g + 1 < NG:
                    emit_z(g + 1)
                z_bc = z_bcs.pop(g)
                # attn_T = (kv_h^T q_h) * z, head pairs share a psum bank
                attn_r = attntp.tile([128, 8, 512], F32R)
                for cc in range(8):
                    aps = attnpsp.tile([128, 512], F32)
                    nc.tensor.matmul(aps[:], lhsT=kv_bd[:, cc, :],
                                     rhs=qT_g[g][:, cc, :], start=True, stop=True)
                    attn_tmp = p2w.tile([128, 512], F32, tag="attn_tmp", name=f"attn_tmp{g}_{cc}")
                    nc.vector.tensor_tensor(out=attn_tmp[:], in0=aps[:],
                                            in1=z_bc[:, cc, :], op=ALU.mult)
                    nc.sync.dma_start(out=attn_r[:, cc, :], in_=attn_tmp[:].bitcast(F32R))

                # proj + residual -> x1; LN2; transpose -> h2T group tile
                h2T = big.tile([128, 8, 512], F32R, tag="grp", name=f"h2T_g{g}")
                for tl in range(4):
                    tt = g * 4 + tl
                    x_rel = p2w.tile([128, C], F32, tag="xrel")
                    nc.sync.dma_start(out=x_rel[:], in_=xs_v[tt])
                    x1_t = p2w.tile([128, C], F32, tag="x1")
                    for oc in range(2):
                        pps = projpsp.tile([128, 512], F32)
                        for cc in range(8):
                            nc.tensor.matmul(pps[:], lhsT=attn_r[:, cc, tl * 128:(tl + 1) * 128],
                                             rhs=wp_sb[:, cc, oc * 512:(oc + 1) * 512],
                                             start=(cc == 0), stop=(cc == 7))
                        osl = slice(oc * 512, (oc + 1) * 512)
                        nc.vector.tensor_tensor(out=x1_t[:, osl], in0=pps[:],
                                                in1=x_rel[:, osl], op=ALU.add)
                        if has_bproj:
                            nc.vector.tensor_tensor(out=x1_t[:, osl], in0=x1_t[:, osl],
                                                    in1=bp_bc[:, osl], op=ALU.add)
                    nc.sync.dma_start(out=x1s[tt], in_=x1_t[:])
                    h2_t = p2w1.tile([128, C], F32, tag="h2")
                    _emit_ln(nc, pools, x1_t, eps_ln_t, h2_t)
                    tr_ps2 = trps2p.tile([128, 8, 128], F32)
                    for cc in range(8):
                        nc.tensor.transpose(tr_ps2[:, cc, :], h2_t[:, cc * 128:(cc + 1) * 128], ident[:])
                    tr_tmp2 = p2w1.tile([128, 8, 128], F32, tag="tr2")
                    nc.vector.tensor_copy(out=tr_tmp2[:], in_=tr_ps2[:])
                    nc.sync.dma_start(out=h2T[:, :, tl * 128:(tl + 1) * 128],
                                        in_=tr_tmp2[:].bitcast(F32R))
                qT_g[g] = h2T  # slot reuse: qT_g[g] fully consumed above

        h2T_g = qT_g  # now holds h2T group tiles

        # ---------------- Phase 3: first half of w2 prefetched in fc1 ----------------
        w2_v = w2.ap().rearrange("(hc p) o -> p hc o", p=128).bitcast(F32R)
        w2pa_cm = tc.tile_pool(name="w2pa", bufs=1)
        w2pa = w2pa_cm.__enter__()
        w2_sba = w2pa.tile([128, 16, C], F32R)
        nc.sync.dma_start(out=w2_sba[:], in_=w2_v[:, 0:16, :])

        # ---------------- Phase 3a: fc1 + gelu -> h3s (DRAM) ----------------
        with tc.tile_pool(name="w1p", bufs=2) as w1p, \
             tc.tile_pool(name="gelt", bufs=2) as geltp, \
             tc.tile_pool(name="f1_ps", bufs=4, space="PSUM") as f1psp:
            w1_v = w1.ap().rearrange("(cc p) o -> p cc o", p=128)
            for hd in range(32):
                w1_col = w1p.tile([128, 8, 128], F32R)
                nc.sync.dma_start(out=w1_col[:],
                                    in_=w1_v[:, :, hd * 128:(hd + 1) * 128].bitcast(F32R))
                for g in range(NG):
                    ps = f1psp.tile([128, 512], F32)
                    for cc in range(8):
                        nc.tensor.matmul(ps[:], lhsT=w1_col[:, cc, :],
                                         rhs=h2T_g[g][:, cc, :],
                                         start=(cc == 0), stop=(cc == 7))
                    gt = geltp.tile([128, 512], F32)
                    nc.scalar.activation(out=gt[:], in_=ps[:], func=AF.Gelu,
                                         bias=bg_sb[:, hd:hd + 1], scale=1.0)
                    nc.sync.dma_start(out=h3s[hd, :, g * 512:(g + 1) * 512], in_=gt[:])

        big_cm.__exit__(None, None, None)

        # ---------------- Phase 3b: fc2 + residual -> out ----------------
        with tc.tile_pool(name="w2pb", bufs=1) as w2pb, \
             tc.tile_pool(name="h3c", bufs=3) as h3cp, \
             tc.tile_pool(name="outp", bufs=2) as outp, \
             tc.tile_pool(name="f2_ps", bufs=3, space="PSUM") as f2psp:
            w2_sbb = w2pb.tile([128, 16, C], F32R)
            nc.sync.dma_start(out=w2_sbb[:], in_=w2_v[:, 16:32, :])
            h3s_v = h3s[:].rearrange("hd p t -> p hd t")
            for tt in range(NT):
                ps = f2psp.tile([128, C], F32)
                h3c = h3cp.tile([128, 32, 128], F32R)
                nc.sync.dma_start(out=h3c[:],
                                  in_=h3s_v[:, :, tt * 128:(tt + 1) * 128].bitcast(F32R))
                for hd in range(32):
                    w2sl = w2_sba[:, hd, :] if hd < 16 else w2_sbb[:, hd - 16, :]
                    for oc in range(2):
                        nc.tensor.matmul(ps[:, oc * 512:(oc + 1) * 512], lhsT=h3c[:, hd, :],
                                         rhs=w2sl[:, oc * 512:(oc + 1) * 512],
                                         start=(hd == 0), stop=(hd == 31))
                x1_rel = outp.tile([128, C], F32, tag="x1rel")
                nc.sync.dma_start(out=x1_rel[:], in_=x1s[tt])
                o_t = outp.tile([128, C], F32, tag="ot")
                nc.vector.tensor_tensor(out=o_t[:], in0=ps[:], in1=x1_rel[:], op=ALU.add)
                if has_bfc2:
                    nc.vector.tensor_tensor(out=o_t[:], in0=o_t[:], in1=b2_bc[:], op=ALU.add)
                nc.sync.dma_start(out=out_v[tt], in_=o_t[:])
        w2pa_cm.__exit__(None, None, None)

    nc.compile()
    return nc


def _prep_inputs(x, norm1_g, norm1_b, qkv_w, proj_w, proj_b, norm2_g, norm2_b,
                 fc1_w, fc1_b, fc2_w, fc2_b):
    """Host-side weight prep. Folds LN gains into weights; LN biases into
    per-output biases. Returns (flags, per-core in_maps)."""
    x = np.asarray(x, np.float32)
    g1 = np.asarray(norm1_g, np.float32)
    b1 = np.asarray(norm1_b, np.float32)
    qkv_w = np.asarray(qkv_w, np.float32)
    proj_w = np.asarray(proj_w, np.float32)
    proj_b = np.asarray(proj_b, np.float32)
    g2 = np.asarray(norm2_g, np.float32)
    b2 = np.asarray(norm2_b, np.float32)
    fc1_w = np.asarray(fc1_w, np.float32)
    fc1_b = np.asarray(fc1_b, np.float32)
    fc2_w = np.asarray(fc2_w, np.float32)
    fc2_b = np.asarray(fc2_b, np.float32)

    wq_t = np.ascontiguousarray((qkv_w[0:C] * g1[None, :]).T)            # [c, o]
    wkv_t = np.ascontiguousarray((qkv_w[C:3 * C] * g1[None, :]).T)       # [c, 2C]
    wp_t = np.ascontiguousarray(proj_w.T)
    w1_t = np.ascontiguousarray((fc1_w * g2[None, :]).T)                 # [c, HID]
    w2_t = np.ascontiguousarray(fc2_w.T)                                 # [HID, c]
    bq_v = qkv_w[0:C] @ b1
    bk_v = qkv_w[C:2 * C] @ b1
    bv_v = qkv_w[2 * C:3 * C] @ b1
    bg_v = fc1_w @ b2 + fc1_b

    flags = (bool(np.any(bk_v)), bool(np.any(bv_v)),
             bool(np.any(proj_b)), bool(np.any(fc2_b)))

    shared = dict(wq=wq_t, wkv=wkv_t, wp=wp_t, w1=w1_t, w2=w2_t,
                  bq=np.ascontiguousarray(bq_v, dtype=np.float32),
                  bk=np.ascontiguousarray(bk_v, dtype=np.float32),
                  bv=np.ascontiguousarray(bv_v, dtype=np.float32),
                  bg=np.ascontiguousarray(bg_v, dtype=np.float32),
                  bp=proj_b, b2o=fc2_b)
    in_maps = []
    for core in range(8):
        b, half = core // 2, core % 2
        xs = np.ascontiguousarray(x[b, half * TOK:(half + 1) * TOK, :])
        in_maps.append({"xs": xs, **shared})
    return flags, in_maps


def get_compiled(flags):
    if flags not in _BUILD_CACHE:
        _BUILD_CACHE[flags] = _build(flags)
    return _BUILD_CACHE[flags]


def kernel(**inputs) -> np.ndarray:
    flags, in_maps = _prep_inputs(**inputs)
    nc = get_compiled(flags)
    res = run_bass_kernel_spmd(nc, in_maps=in_maps, core_ids=list(range(8)))
    shards = [res.results[c]["out"] for c in range(8)]
    full = np.empty((B, N, C), np.float32)
    for core in range(8):
        b, half = core // 2, core % 2
        full[b, half * TOK:(half + 1) * TOK, :] = shards[core]
    return full


# revision 26
# speedup vs baseline: 1.0333x; 1.0333x over previous
"""Trainium2 Bass kernel for nn_Block_9457517985872 (dense transformer block
with linear attention). Token-sharded across 8 NeuronCores: core c handles
batch c//2, sequence half c%2 (2048 tokens). Only cross-core communication is
a pairwise AllReduce of the per-head (kv, ksum) statistics [16,64,65] f32.

Self-contained: hardcodes all shapes from the problem spec.
"""
import numpy as np
from contextlib import ExitStack

import concourse.bass as bass
import concourse.tile as tile
from concourse import bacc, mybir
from concourse.bass_utils import run_bass_kernel_spmd
from concourse.masks import make_identity

F32 = mybir.dt.float32
F32R = mybir.dt.float32r
AF = mybir.ActivationFunctionType
ALU = mybir.AluOpType

B, N, C = 4, 4096, 1024
H, D = 16, 64
HID = 4096
TOK = 2048          # tokens per core
NT = TOK // 128     # 16 token tiles
NG = TOK // 512     # 4 token groups
EPS_LN = 1e-5
EPS_ATTN = 1e-6

_BUILD_CACHE = {}


def _emit_ln(nc, pools, x_t, eps_t, out_t):
    """LayerNorm core (no gamma/beta): out = (x - mean(x)) * rsqrt(var + eps).
    x_t: [128, 1024] f32 SBUF tile. out_t may alias x_t."""
    stats = pools["stat"].tile([128, 2, 6], F32, name="ln_stats", tag="ln_stats")
    mv = pools["stat"].tile([128, 2], F32, name="ln_mv", tag="ln_mv")
    for sg in range(2):
        nc.vector.bn_stats(out=stats[:, sg, :], in_=x_t[:, sg * 512:(sg + 1) * 512])
    nc.vector.bn_aggr(out=mv[:], in_=stats[:])
    # mv[:,0]=mean, mv[:,1]=var -> rstd
    nc.scalar.activation(out=mv[:, 1:2], in_=mv[:, 1:2], func=AF.Sqrt, bias=eps_t[:], scale=1.0)
    nc.vector.reciprocal(out=mv[:, 1:2], in_=mv[:, 1:2])
    # mv[:,0] = -mean*rstd
    nc.vector.tensor_tensor(out=mv[:, 0:1], in0=mv[:, 0:1], in1=mv[:, 1:2], op=ALU.mult)
    nc.vector.tensor_scalar_mul(out=mv[:, 0:1], in0=mv[:, 0:1], scalar1=-1.0)
    nc.scalar.activation(out=out_t[:], in_=x_t[:], func=AF.Identity,
                         bias=mv[:, 0:1], scale=mv[:, 1:2])


def _build(flags, no_cc=False):
    """flags: (has_bk, has_bv, has_bproj, has_bfc2)"""
    has_bk, has_bv, has_bproj, has_bfc2 = flags
    nc = bacc.Bacc("TRN2", target_bir_lowering=False, debug=False,
                   num_devices=1 if no_cc else 8)

    xs = nc.dram_tensor("xs", [TOK, C], F32, kind="ExternalInput")
    wq = nc.dram_tensor("wq", [C, C], F32, kind="ExternalInput")      # [c, o]
    wkv = nc.dram_tensor("wkv", [C, 2 * C], F32, kind="ExternalInput")
    wp = nc.dram_tensor("wp", [C, C], F32, kind="ExternalInput")
    w1 = nc.dram_tensor("w1", [C, HID], F32, kind="ExternalInput")
    w2 = nc.dram_tensor("w2", [HID, C], F32, kind="ExternalInput")
    bq = nc.dram_tensor("bq", [C], F32, kind="ExternalInput")
    bk = nc.dram_tensor("bk", [C], F32, kind="ExternalInput")
    bv = nc.dram_tensor("bv", [C], F32, kind="ExternalInput")
    bg = nc.dram_tensor("bg", [HID], F32, kind="ExternalInput")
    bp = nc.dram_tensor("bp", [C], F32, kind="ExternalInput")
    b2o = nc.dram_tensor("b2o", [C], F32, kind="ExternalInput")
    out = nc.dram_tensor("out", [TOK, C], F32, kind="ExternalOutput")

    xs_v = xs.ap().rearrange("(t p) c -> t p c", p=128)     # [16,128,1024]
    out_v = out.ap().rearrange("(t p) c -> t p c", p=128)

    with tile.TileContext(nc) as tc, ExitStack() as ctx:
        const = ctx.enter_context(tc.tile_pool(name="const", bufs=1))
        dram = ctx.enter_context(tc.tile_pool(name="dram", bufs=1, space="DRAM"))
        statp = ctx.enter_context(tc.tile_pool(name="stat", bufs=4))
        pools = {"stat": statp}

        ident = const.tile([128, 128], F32)
        make_identity(nc, ident[:])
        eps_ln_t = const.tile([128, 1], F32)
        nc.vector.memset(eps_ln_t[:], EPS_LN)
        bq_sb = const.tile([128, 8], F32)
        nc.sync.dma_start(out=bq_sb[:], in_=bq.ap().rearrange("(oc p) -> p oc", p=128))
        bg_sb = const.tile([128, 32], F32)
        nc.sync.dma_start(out=bg_sb[:], in_=bg.ap().rearrange("(hd p) -> p hd", p=128))
        if has_bk:
            bk_bc = const.tile([128, C], F32)
            nc.sync.dma_start(out=bk_bc[:], in_=bass.AP(
                tensor=bk.ap().tensor, offset=0, ap=[[0, 128], [1, C]]))
        if has_bproj:
            bp_bc = const.tile([128, C], F32)
            nc.sync.dma_start(out=bp_bc[:], in_=bass.AP(
                tensor=bp.ap().tensor, offset=0, ap=[[0, 128], [1, C]]))
        if has_bfc2:
            b2_bc = const.tile([128, C], F32)
            nc.sync.dma_start(out=b2_bc[:], in_=bass.AP(
                tensor=b2o.ap().tensor, offset=0, ap=[[0, 128], [1, C]]))

        x1s = dram.tile([NT, 128, C], F32)
        h3s = dram.tile([32, 128, TOK], F32)
        cci = dram.tile([2, 128, 4, 65], F32)
        cco = dram.tile([2, 128, 4, 65], F32)
        z_d = dram.tile([16, TOK], F32)
        ht_d = dram.tile([NT, 128, 8, 128], F32)

        # ---------------- Phase 1a: LN1, hT, k/v, kv+ksum ----------------
        with (
            tc.tile_pool(name="wkvp", bufs=1) as wkvp,
            tc.tile_pool(name="p1w", bufs=3) as p1w,
            tc.tile_pool(name="htrp", bufs=2) as htrp,
            tc.tile_pool(name="trtmp", bufs=2) as trtmpp,
            tc.tile_pool(name="kvstage", bufs=1) as kvstagep,
            tc.tile_pool(name="kvacc_ps", bufs=1, space="PSUM") as kvaccp,
            tc.tile_pool(name="tr_ps", bufs=1, space="PSUM") as trpsp,
            tc.tile_pool(name="gen_ps", bufs=3, space="PSUM") as genpsp,
        ):
            wkv_sb = wkvp.tile([128, 8, 2 * C], F32R)
            wkv_v = wkv.ap().rearrange("(cc p) o -> p cc o", p=128).bitcast(F32R)
            for cc in range(8):
                nc.sync.dma_start(out=wkv_sb[:, cc, :], in_=wkv_v[:, cc, :])
            kv_ps = [kvaccp.tile([128, 4, 65], F32, name=f"kv_ps{i}") for i in range(2)]

            for tt in range(NT):
                x_t = p1w.tile([128, C], F32, tag="x")
                nc.sync.dma_start(out=x_t[:], in_=xs_v[tt])
                _emit_ln(nc, pools, x_t, eps_ln_t, x_t)
                # transpose h (=x_t) -> hT_full[:, :, tt*128:+128]
                tr_ps = trpsp.tile([128, 8, 128], F32)
                for cc in range(8):
                    nc.tensor.transpose(tr_ps[:, cc, :], x_t[:, cc * 128:(cc + 1) * 128], ident[:])
                tr_tmp = trtmpp.tile([128, 8, 128], F32)
                nc.vector.tensor_copy(out=tr_tmp[:], in_=tr_ps[:])
                nc.sync.dma_start(out=ht_d[tt], in_=tr_tmp[:])
                hT_r = htrp.tile([128, 8, 128], F32R)
                nc.sync.dma_start(out=hT_r[:], in_=tr_tmp[:].bitcast(F32R))
                # k, v for this tile
                k_sb = p1w.tile([128, C], F32, tag="k")
                v_ext = p1w.tile([128, H, 65], F32, tag="v")
                nc.vector.memset(v_ext[:, :, 64:65], 1.0)
                for oc in range(4):
                    ps = genpsp.tile([128, 512], F32, tag="gen")
                    for cc in range(8):
                        nc.tensor.matmul(ps[:], lhsT=hT_r[:, cc, :],
                                         rhs=wkv_sb[:, cc, oc * 512:(oc + 1) * 512],
                                         start=(cc == 0), stop=(cc == 7))
                    if oc < 2:  # k: phi = exp(min(x,0)) + relu(x)
                        ksl = k_sb[:, oc * 512:(oc + 1) * 512]
                        if has_bk:
                            nc.vector.tensor_tensor(out=ksl, in0=ps[:],
                                                    in1=bk_bc[:, oc * 512:(oc + 1) * 512], op=ALU.add)
                            src = ksl
                        else:
                            src = ps[:]
                        mt = p1w.tile([128, 512], F32, tag="phim")
                        nc.vector.tensor_scalar_min(out=mt[:], in0=src, scalar1=0.0)
                        nc.scalar.activation(out=mt[:], in_=mt[:], func=AF.Exp)
                        nc.vector.scalar_tensor_tensor(out=ksl, in0=src, scalar=0.0,
                                                       in1=mt[:], op0=ALU.max, op1=ALU.add)
                    else:      # v -> v_ext[:, heads, 0:64]
                        h0 = (oc - 2) * 8
                        dst = v_ext[:, h0:h0 + 8, 0:64]
                        if has_bv:
                            vb = bass.AP(tensor=bv.ap().tensor, offset=(oc - 2) * 512,
                                         ap=[[0, 128], [64, 8], [1, 64]])
                            vb_t = p1w.tile([128, 8, 64], F32, tag="vb")
                            nc.sync.dma_start(out=vb_t[:], in_=vb)
                            nc.vector.tensor_tensor(
                                out=dst, in0=ps[:].rearrange("p (h d) -> p h d", d=64),
                                in1=vb_t[:], op=ALU.add)
                        else:
                            nc.vector.tensor_copy(
                                out=dst, in_=ps[:].rearrange("p (h d) -> p h d", d=64))
                # kv accumulation: per head [64, 65] += k_h^T @ [v_h | 1]
                for h in range(H):
                    ti, hf, slot = h // 8, (h % 8) // 4, h % 4
                    nc.tensor.matmul(
                        kv_ps[ti][hf * 64:(hf + 1) * 64, slot, :],
                        lhsT=k_sb[:, h * 64:(h + 1) * 64],
                        rhs=v_ext[:, h, :],
                        start=(tt == 0), stop=(tt == NT - 1))

            # stage kv psum -> SBUF -> DRAM, then pairwise AllReduce
            kv_st = kvstagep.tile([128, 2, 4, 65], F32)
            for ti in range(2):
                nc.vector.tensor_copy(out=kv_st[:, ti], in_=kv_ps[ti][:])
                nc.sync.dma_start(out=cci[ti], in_=kv_st[:, ti])
            if no_cc:
                nc.sync.dma_start(out=cco[:], in_=cci[:])
            else:
                nc.gpsimd.collective_compute(
                    "AllReduce", ALU.add,
                    replica_groups=[[0, 1], [2, 3], [4, 5], [6, 7]],
                    ins=[cci[:]], outs=[cco[:]])

        # ------------- Phase 1b: qT (overlaps the collective) -------------
        big_cm = tc.tile_pool(name="big", bufs=4, side="right")
        big = big_cm.__enter__()
        qT_g = [big.tile([128, 8, 512], F32R, tag="grp", name=f"qT_g{i}") for i in range(NG)]
        with tc.tile_pool(name="wqp", bufs=2) as wqp, \
             tc.tile_pool(name="p1bw", bufs=3) as p1bw, \
             tc.tile_pool(name="qhtp", bufs=2) as qhtp, \
             tc.tile_pool(name="q_ps", bufs=4, space="PSUM") as qpsp:
            for g in range(NG):
                qht = qhtp.tile([128, 8, 4, 128], F32R)
                nc.sync.dma_start(out=qht[:], in_=ht_d[4 * g:4 * (g + 1)].rearrange(
                    "tl p cc t -> p cc tl t").bitcast(F32R))
                qht_v = qht[:].rearrange("p cc tl t -> p cc (tl t)")
                for oc in range(8):
                    wq_col = wqp.tile([128, 8, 128], F32R)
                    nc.sync.dma_start(out=wq_col[:], in_=wq.ap().rearrange(
                        "(cc p) o -> p cc o", p=128)[:, :, oc * 128:(oc + 1) * 128].bitcast(F32R))
                    ps = qpsp.tile([128, 512], F32)
                    for cc in range(8):
                        nc.tensor.matmul(ps[:], lhsT=wq_col[:, cc, :],
                                         rhs=qht_v[:, cc, :],
                                         start=(cc == 0), stop=(cc == 7))
                    mt = p1bw.tile([128, 512], F32, tag="phim")
                    rt = p1bw.tile([128, 512], F32, tag="phir")
                    nc.vector.tensor_scalar(out=mt[:], in0=ps[:], scalar1=bq_sb[:, oc:oc + 1],
                                            scalar2=0.0, op0=ALU.add, op1=ALU.min)
                    nc.scalar.activation(out=mt[:], in_=mt[:], func=AF.Exp)
                    nc.vector.tensor_scalar(out=rt[:], in0=ps[:], scalar1=bq_sb[:, oc:oc + 1],
                                            scalar2=0.0, op0=ALU.add, op1=ALU.max)
                    nc.vector.tensor_tensor(out=mt[:], in0=mt[:], in1=rt[:], op=ALU.add)
                    nc.sync.dma_start(out=qT_g[g][:, oc, :], in_=mt[:].bitcast(F32R))

        # ---------------- Phase 2: attention + proj + LN2 ----------------
        with (
            tc.tile_pool(name="wpp", bufs=1) as wpp,
            tc.tile_pool(name="kv2", bufs=1) as kv2p,
            tc.tile_pool(name="p2w", bufs=2) as p2w,
            tc.tile_pool(name="p2w1", bufs=3) as p2w1,
            tc.tile_pool(name="attnt", bufs=1) as attntp,
            tc.tile_pool(name="zbcpa", bufs=1) as zbcpa,
            tc.tile_pool(name="z_ps", bufs=2, space="PSUM") as zpsp,
            tc.tile_pool(name="attn_ps", bufs=2, space="PSUM") as attnpsp,
            tc.tile_pool(name="proj_ps", bufs=2, space="PSUM") as projpsp,
            tc.tile_pool(name="tr2_ps", bufs=1, space="PSUM") as trps2p,
        ):
            wp_sb = wpp.tile([128, 8, C], F32R)
            nc.sync.dma_start(out=wp_sb[:], in_=wp.ap().rearrange(
                "(cc p) o -> p cc o", p=128).bitcast(F32R))
            kv_sb2 = kv2p.tile([128, 8, 65], F32R)
            kv_bd = kv2p.tile([128, 8, 128], F32R)
            nc.vector.memset(kv_bd[:].bitcast(F32), 0.0)
            bd = kv2p.tile([128, 8, 16], F32R)
            nc.vector.memset(bd[:].bitcast(F32), 0.0)
            for h in range(H):
                ti, hf, slot = h // 8, (h % 8) // 4, h % 4
                pbase = (h % 2) * 64
                nc.sync.dma_start(
                    out=kv_sb2[pbase:pbase + 64, h // 2, :],
                    in_=cco[ti, hf * 64:(hf + 1) * 64, slot, :].bitcast(F32R))
                # block-diagonal kv per head pair: head h occupies rows/cols
                # [pbase, pbase+64) of kv_bd[:, h//2, :]
                nc.sync.dma_start(
                    out=kv_bd[pbase:pbase + 64, h // 2, pbase:pbase + 64],
                    in_=kv_sb2[pbase:pbase + 64, h // 2, 0:64])
                nc.sync.dma_start(
                    out=bd[pbase:pbase + 64, h // 2, h:h + 1],
                    in_=kv_sb2[pbase:pbase + 64, h // 2, 64:65])
            z_bcs = {}

            def emit_z(g):
                # z = 1 / (q . ksum + eps), then broadcast to head-pair layout
                zps = zpsp.tile([16, 512], F32, name=f"zps{g}", tag="zps")
                for pc in range(8):
                    nc.tensor.matmul(zps[:], lhsT=bd[:, pc, :], rhs=qT_g[g][:, pc, :],
                                     start=(pc == 0), stop=(pc == 7))
                zsl = p2w.tile([16, 512], F32, name=f"zt{g}", tag="zt")
                nc.vector.tensor_scalar_add(out=zsl[:], in0=zps[:], scalar1=EPS_ATTN)
                nc.vector.reciprocal(out=zsl[:], in_=zsl[:])
                nc.sync.dma_start(out=z_d[:, g * 512:(g + 1) * 512], in_=zsl[:])
                z_bc = zbcpa.tile([128, 8, 512], F32, name=f"zbc{g}", tag="zbc")
                zd_ap = z_d[:]
                for sub in range(2):
                    nc.sync.dma_start(
                        out=z_bc[sub * 64:(sub + 1) * 64, :, :],
                        in_=bass.AP(tensor=zd_ap.tensor,
                                    offset=zd_ap.offset + sub * TOK + g * 512,
                                    ap=[[0, 64], [2 * TOK, 8], [1, 512]]))
                z_bcs[g] = z_bc

            emit_z(0)
            for g in range(NG):
                if g + 1 < NG:
                    emit_z(g + 1)
                z_bc = z_bcs.pop(g)
                # attn_T = (kv_h^T q_h) * z, head pairs share a psum bank
                attn_r = attntp.tile([128, 8, 512], F32R)
                for cc in range(8):
                    aps = attnpsp.tile([128, 512], F32)
                    nc.tensor.matmul(aps[:], lhsT=kv_bd[:, cc, :],
                                     rhs=qT_g[g][:, cc, :], start=True, stop=True)
                    attn_tmp = p2w.tile([128, 512], F32, tag="attn_tmp", name=f"attn_tmp{g}_{cc}")
                    nc.vector.tensor_tensor(out=attn_tmp[:], in0=aps[:],
                                            in1=z_bc[:, cc, :], op=ALU.mult)
                    nc.sync.dma_start(out=attn_r[:, cc, :], in_=attn_tmp[:].bitcast(F32R))

                # proj + residual -> x1; LN2; transpose -> h2T group tile
                h2T = big.tile([128, 8, 512], F32R, tag="grp", name=f"h2T_g{g}")
                for tl in range(4):
                    tt = g * 4 + tl
                    x_rel = p2w.tile([128, C], F32, tag="xrel")
                    nc.sync.dma_start(out=x_rel[:], in_=xs_v[tt])
                    x1_t = p2w.tile([128, C], F32, tag="x1")
                    for oc in range(2):
                        pps = projpsp.tile([128, 512], F32)
                        for cc in range(8):
                            nc.tensor.matmul(pps[:], lhsT=attn_r[:, cc, tl * 128:(tl + 1) * 128],
                                             rhs=wp_sb[:, cc, oc * 512:(oc + 1) * 512],
                                             start=(cc == 0), stop=(cc == 7))
                        osl = slice(oc * 512, (oc + 1) * 512)
                        nc.vector.tensor_tensor(out=x1_t[:, osl], in0=pps[:],
                                                in1=x_rel[:, osl], op=ALU.add)
                        if has_bproj:
                            nc.vector.tensor_tensor(out=x1_t[:, osl], in0=x1_t[:, osl],
                                                    in1=bp_bc[:, osl], op=ALU.add)
                    nc.sync.dma_start(out=x1s[tt], in_=x1_t[:])
                    h2_t = p2w1.tile([128, C], F32, tag="h2")
                    _emit_ln(nc, pools, x1_t, eps_ln_t, h2_t)
                    tr_ps2 = trps2p.tile([128, 8, 128], F32)
                    for cc in range(8):
                        nc.tensor.transpose(tr_ps2[:, cc, :], h2_t[:, cc * 128:(cc + 1) * 128], ident[:])
                    tr_tmp2 = p2w1.tile([128, 8, 128], F32, tag="tr2")
                    nc.vector.tensor_copy(out=tr_tmp2[:], in_=tr_ps2[:])
                    nc.sync.dma_start(out=h2T[:, :, tl * 128:(tl + 1) * 128],
                                        in_=tr_tmp2[:].bitcast(F32R))
                qT_g[g] = h2T  # slot reuse: qT_g[g] fully consumed above

        h2T_g = qT_g  # now holds h2T group tiles

        w2_v = w2.ap().rearrange("(hc p) o -> p hc o", p=128).bitcast(F32R)

        # ---------------- Phase 3a: fc1 + gelu -> h3s (DRAM) ----------------
        with tc.tile_pool(name="w1p", bufs=2) as w1p, \
             tc.tile_pool(name="gelt", bufs=2) as geltp, \
             tc.tile_pool(name="f1_ps", bufs=4, space="PSUM") as f1psp:
            w1_v = w1.ap().rearrange("(cc p) o -> p cc o", p=128)
            for hd in range(32):
                w1_col = w1p.tile([128, 8, 128], F32R)
                nc.sync.dma_start(out=w1_col[:],
                                    in_=w1_v[:, :, hd * 128:(hd + 1) * 128].bitcast(F32R))
                for g in range(NG):
                    ps = f1psp.tile([128, 512], F32)
                    for cc in range(8):
                        nc.tensor.matmul(ps[:], lhsT=w1_col[:, cc, :],
                                         rhs=h2T_g[g][:, cc, :],
                                         start=(cc == 0), stop=(cc == 7))
                    gt = geltp.tile([128, 512], F32)
                    nc.scalar.activation(out=gt[:], in_=ps[:], func=AF.Gelu,
                                         bias=bg_sb[:, hd:hd + 1], scale=1.0)
                    nc.sync.dma_start(out=h3s[hd, :, g * 512:(g + 1) * 512], in_=gt[:])

        big_cm.__exit__(None, None, None)

        # ---------------- Phase 3b: fc2 + residual -> out ----------------
        with tc.tile_pool(name="w2p", bufs=1) as w2p, \
             tc.tile_pool(name="h3c", bufs=3) as h3cp, \
             tc.tile_pool(name="outp", bufs=2) as outp, \
             tc.tile_pool(name="f2_ps", bufs=3, space="PSUM") as f2psp:
            w2_sb = w2p.tile([128, 32, C], F32R)
            for hc in range(8):
                nc.sync.dma_start(out=w2_sb[:, 4 * hc:4 * (hc + 1), :],
                                  in_=w2_v[:, 4 * hc:4 * (hc + 1), :])
            h3s_v = h3s[:].rearrange("hd p t -> p hd t")
            for tt in range(NT):
                ps = f2psp.tile([128, C], F32)
                h3c = h3cp.tile([128, 32, 128], F32R)
                nc.sync.dma_start(out=h3c[:],
                                  in_=h3s_v[:, :, tt * 128:(tt + 1) * 128].bitcast(F32R))
                for hd in range(32):
                    for oc in range(2):
                        nc.tensor.matmul(ps[:, oc * 512:(oc + 1) * 512], lhsT=h3c[:, hd, :],
                                         rhs=w2_sb[:, hd, oc * 512:(oc + 1) * 512],
                                         start=(hd == 0), stop=(hd == 31))
                x1_rel = outp.tile([128, C], F32, tag="x1rel")
                nc.sync.dma_start(out=x1_rel[:], in_=x1s[tt])
                o_t = outp.tile([128, C], F32, tag="ot")
                nc.vector.tensor_tensor(out=o_t[:], in0=ps[:], in1=x1_rel[:], op=ALU.add)
                if has_bfc2:
                    nc.vector.tensor_tensor(out=o_t[:], in0=o_t[:], in1=b2_bc[:], op=ALU.add)
                nc.sync.dma_start(out=out_v[tt], in_=o_t[:])

    nc.compile()
    return nc


def _prep_inputs(x, norm1_g, norm1_b, qkv_w, proj_w, proj_b, norm2_g, norm2_b,
                 fc1_w, fc1_b, fc2_w, fc2_b):
    """Host-side weight prep. Folds LN gains into weights; LN biases into
    per-output biases. Returns (flags, per-core in_maps)."""
    x = np.asarray(x, np.float32)
    g1 = np.asarray(norm1_g, np.float32)
    b1 = np.asarray(norm1_b, np.float32)
    qkv_w = np.asarray(qkv_w, np.float32)
    proj_w = np.asarray(proj_w, np.float32)
    proj_b = np.asarray(proj_b, np.float32)
    g2 = np.asarray(norm2_g, np.float32)
    b2 = np.asarray(norm2_b, np.float32)
    fc1_w = np.asarray(fc1_w, np.float32)
    fc1_b = np.asarray(fc1_b, np.float32)
    fc2_w = np.asarray(fc2_w, np.float32)
    fc2_b = np.asarray(fc2_b, np.float32)

    wq_t = np.ascontiguousarray((qkv_w[0:C] * g1[None, :]).T)            # [c, o]
    wkv_t = np.ascontiguousarray((qkv_w[C:3 * C] * g1[None, :]).T)       # [c, 2C]
    wp_t = np.ascontiguousarray(proj_w.T)
    w1_t = np.ascontiguousarray((fc1_w * g2[None, :]).T)                 # [c, HID]
    w2_t = np.ascontiguousarray(fc2_w.T)                                 # [HID, c]
    bq_v = qkv_w[0:C] @ b1
    bk_v = qkv_w[C:2 * C] @ b1
    bv_v = qkv_w[2 * C:3 * C] @ b1
    bg_v = fc1_w @ b2 + fc1_b

    flags = (bool(np.any(bk_v)), bool(np.any(bv_v)),
             bool(np.any(proj_b)), bool(np.any(fc2_b)))

    shared = dict(wq=wq_t, wkv=wkv_t, wp=wp_t, w1=w1_t, w2=w2_t,
                  bq=np.ascontiguousarray(bq_v, dtype=np.float32),
                  bk=np.ascontiguousarray(bk_v, dtype=np.float32),
                  bv=np.ascontiguousarray(bv_v, dtype=np.float32),
                  bg=np.ascontiguousarray(bg_v, dtype=np.float32),
                  bp=proj_b, b2o=fc2_b)
    in_maps = []
    for core in range(8):
        b, half = core // 2, core % 2
        xs = np.ascontiguousarray(x[b, half * TOK:(half + 1) * TOK, :])
        in_maps.append({"xs": xs, **shared})
    return flags, in_maps


def get_compiled(flags):
    if flags not in _BUILD_CACHE:
        _BUILD_CACHE[flags] = _build(flags)
    return _BUILD_CACHE[flags]


def kernel(**inputs) -> np.ndarray:
    flags, in_maps = _prep_inputs(**inputs)
    nc = get_compiled(flags)
    res = run_bass_kernel_spmd(nc, in_maps=in_maps, core_ids=list(range(8)))
    shards = [res.results[c]["out"] for c in range(8)]
    full = np.empty((B, N, C), np.float32)
    for core in range(8):
        b, half = core // 2, core % 2
        full[b, half * TOK:(half + 1) * TOK, :] = shards[core]
    return full


# revision 28
# speedup vs baseline: 1.0423x; 1.0088x over previous
"""Trainium2 Bass kernel for nn_Block_9457517985872 (dense transformer block
with linear attention). Token-sharded across 8 NeuronCores: core c handles
batch c//2, sequence half c%2 (2048 tokens). Only cross-core communication is
a pairwise AllReduce of the per-head (kv, ksum) statistics [16,64,65] f32.

Self-contained: hardcodes all shapes from the problem spec.
"""
import numpy as np
from contextlib import ExitStack

import concourse.bass as bass
import concourse.tile as tile
from concourse import bacc, mybir
from concourse.bass_utils import run_bass_kernel_spmd
from concourse.masks import make_identity

F32 = mybir.dt.float32
F32R = mybir.dt.float32r
AF = mybir.ActivationFunctionType
ALU = mybir.AluOpType

B, N, C = 4, 4096, 1024
H, D = 16, 64
HID = 4096
TOK = 2048          # tokens per core
NT = TOK // 128     # 16 token tiles
NG = TOK // 512     # 4 token groups
EPS_LN = 1e-5
EPS_ATTN = 1e-6

_BUILD_CACHE = {}


def _emit_ln(nc, pools, x_t, eps_t, out_t):
    """LayerNorm core (no gamma/beta): out = (x - mean(x)) * rsqrt(var + eps).
    x_t: [128, 1024] f32 SBUF tile. out_t may alias x_t."""
    stats = pools["stat"].tile([128, 2, 6], F32, name="ln_stats", tag="ln_stats")
    mv = pools["stat"].tile([128, 2], F32, name="ln_mv", tag="ln_mv")
    for sg in range(2):
        nc.vector.bn_stats(out=stats[:, sg, :], in_=x_t[:, sg * 512:(sg + 1) * 512])
    nc.vector.bn_aggr(out=mv[:], in_=stats[:])
    # mv[:,0]=mean, mv[:,1]=var -> rstd
    nc.scalar.activation(out=mv[:, 1:2], in_=mv[:, 1:2], func=AF.Sqrt, bias=eps_t[:], scale=1.0)
    nc.vector.reciprocal(out=mv[:, 1:2], in_=mv[:, 1:2])
    # mv[:,0] = -mean*rstd
    nc.vector.tensor_tensor(out=mv[:, 0:1], in0=mv[:, 0:1], in1=mv[:, 1:2], op=ALU.mult)
    nc.vector.tensor_scalar_mul(out=mv[:, 0:1], in0=mv[:, 0:1], scalar1=-1.0)
    nc.scalar.activation(out=out_t[:], in_=x_t[:], func=AF.Identity,
                         bias=mv[:, 0:1], scale=mv[:, 1:2])


def _build(flags, no_cc=False):
    """flags: (has_bk, has_bv, has_bproj, has_bfc2)"""
    has_bk, has_bv, has_bproj, has_bfc2 = flags
    nc = bacc.Bacc("TRN2", target_bir_lowering=False, debug=False,
                   num_devices=1 if no_cc else 8)

    xs = nc.dram_tensor("xs", [TOK, C], F32, kind="ExternalInput")
    wq = nc.dram_tensor("wq", [C, C], F32, kind="ExternalInput")      # [c, o]
    wkv = nc.dram_tensor("wkv", [C, 2 * C], F32, kind="ExternalInput")
    wp = nc.dram_tensor("wp", [C, C], F32, kind="ExternalInput")
    w1 = nc.dram_tensor("w1", [C, HID], F32, kind="ExternalInput")
    w2 = nc.dram_tensor("w2", [HID, C], F32, kind="ExternalInput")
    bq = nc.dram_tensor("bq", [C], F32, kind="ExternalInput")
    bk = nc.dram_tensor("bk", [C], F32, kind="ExternalInput")
    bv = nc.dram_tensor("bv", [C], F32, kind="ExternalInput")
    bg = nc.dram_tensor("bg", [HID], F32, kind="ExternalInput")
    bp = nc.dram_tensor("bp", [C], F32, kind="ExternalInput")
    b2o = nc.dram_tensor("b2o", [C], F32, kind="ExternalInput")
    out = nc.dram_tensor("out", [TOK, C], F32, kind="ExternalOutput")

    xs_v = xs.ap().rearrange("(t p) c -> t p c", p=128)     # [16,128,1024]
    out_v = out.ap().rearrange("(t p) c -> t p c", p=128)

    with tile.TileContext(nc) as tc, ExitStack() as ctx:
        const = ctx.enter_context(tc.tile_pool(name="const", bufs=1))
        dram = ctx.enter_context(tc.tile_pool(name="dram", bufs=1, space="DRAM"))
        statp = ctx.enter_context(tc.tile_pool(name="stat", bufs=4))
        pools = {"stat": statp}

        ident = const.tile([128, 128], F32)
        make_identity(nc, ident[:])
        eps_ln_t = const.tile([128, 1], F32)
        nc.vector.memset(eps_ln_t[:], EPS_LN)
        bq_sb = const.tile([128, 8], F32)
        nc.sync.dma_start(out=bq_sb[:], in_=bq.ap().rearrange("(oc p) -> p oc", p=128))
        bg_sb = const.tile([128, 32], F32)
        nc.sync.dma_start(out=bg_sb[:], in_=bg.ap().rearrange("(hd p) -> p hd", p=128))
        if has_bk:
            bk_bc = const.tile([128, C], F32)
            nc.sync.dma_start(out=bk_bc[:], in_=bass.AP(
                tensor=bk.ap().tensor, offset=0, ap=[[0, 128], [1, C]]))
        if has_bproj:
            bp_bc = const.tile([128, C], F32)
            nc.sync.dma_start(out=bp_bc[:], in_=bass.AP(
                tensor=bp.ap().tensor, offset=0, ap=[[0, 128], [1, C]]))
        if has_bfc2:
            b2_bc = const.tile([128, C], F32)
            nc.sync.dma_start(out=b2_bc[:], in_=bass.AP(
                tensor=b2o.ap().tensor, offset=0, ap=[[0, 128], [1, C]]))

        x1s = dram.tile([NT, 128, C], F32)
        h3s = dram.tile([32, 128, TOK], F32)
        cci = dram.tile([2, 128, 4, 65], F32)
        cco = dram.tile([2, 128, 4, 65], F32)
        z_d = dram.tile([16, TOK], F32)
        ht_d = dram.tile([NT, 128, 8, 128], F32)

        # ---------------- Phase 1a: LN1, hT, k/v, kv+ksum ----------------
        with (
            tc.tile_pool(name="wkvp", bufs=1) as wkvp,
            tc.tile_pool(name="p1w", bufs=3) as p1w,
            tc.tile_pool(name="htrp", bufs=2) as htrp,
            tc.tile_pool(name="trtmp", bufs=2) as trtmpp,
            tc.tile_pool(name="kvstage", bufs=1) as kvstagep,
            tc.tile_pool(name="kvacc_ps", bufs=1, space="PSUM") as kvaccp,
            tc.tile_pool(name="tr_ps", bufs=1, space="PSUM") as trpsp,
            tc.tile_pool(name="gen_ps", bufs=3, space="PSUM") as genpsp,
        ):
            x_tiles = {}
            for tt in range(2):
                x_t = p1w.tile([128, C], F32, tag="x", name=f"x_t{tt}")
                nc.sync.dma_start(out=x_t[:], in_=xs_v[tt])
                x_tiles[tt] = x_t
            wkv_sb = wkvp.tile([128, 8, 2 * C], F32R)
            wkv_v = wkv.ap().rearrange("(cc p) o -> p cc o", p=128).bitcast(F32R)
            for oc in range(4):
                nc.sync.dma_start(out=wkv_sb[:, :, oc * 512:(oc + 1) * 512],
                                  in_=wkv_v[:, :, oc * 512:(oc + 1) * 512])
            kv_ps = [kvaccp.tile([128, 4, 65], F32, name=f"kv_ps{i}") for i in range(2)]

            for tt in range(NT):
                if tt in x_tiles:
                    x_t = x_tiles.pop(tt)
                else:
                    x_t = p1w.tile([128, C], F32, tag="x", name=f"x_t{tt}")
                    nc.sync.dma_start(out=x_t[:], in_=xs_v[tt])
                _emit_ln(nc, pools, x_t, eps_ln_t, x_t)
                # transpose h (=x_t) -> hT_full[:, :, tt*128:+128]
                tr_ps = trpsp.tile([128, 8, 128], F32)
                for cc in range(8):
                    nc.tensor.transpose(tr_ps[:, cc, :], x_t[:, cc * 128:(cc + 1) * 128], ident[:])
                tr_tmp = trtmpp.tile([128, 8, 128], F32)
                nc.vector.tensor_copy(out=tr_tmp[:], in_=tr_ps[:])
                nc.sync.dma_start(out=ht_d[tt], in_=tr_tmp[:])
                hT_r = htrp.tile([128, 8, 128], F32R)
                nc.sync.dma_start(out=hT_r[:], in_=tr_tmp[:].bitcast(F32R))
                # k, v for this tile
                k_sb = p1w.tile([128, C], F32, tag="k")
                v_ext = p1w.tile([128, H, 65], F32, tag="v")
                nc.vector.memset(v_ext[:, :, 64:65], 1.0)
                for oc in range(4):
                    ps = genpsp.tile([128, 512], F32, tag="gen")
                    for cc in range(8):
                        nc.tensor.matmul(ps[:], lhsT=hT_r[:, cc, :],
                                         rhs=wkv_sb[:, cc, oc * 512:(oc + 1) * 512],
                                         start=(cc == 0), stop=(cc == 7))
                    if oc < 2:  # k: phi = exp(min(x,0)) + relu(x)
                        ksl = k_sb[:, oc * 512:(oc + 1) * 512]
                        if has_bk:
                            nc.vector.tensor_tensor(out=ksl, in0=ps[:],
                                                    in1=bk_bc[:, oc * 512:(oc + 1) * 512], op=ALU.add)
                            src = ksl
                        else:
                            src = ps[:]
                        mt = p1w.tile([128, 512], F32, tag="phim")
                        nc.vector.tensor_scalar_min(out=mt[:], in0=src, scalar1=0.0)
                        nc.scalar.activation(out=mt[:], in_=mt[:], func=AF.Exp)
                        nc.vector.scalar_tensor_tensor(out=ksl, in0=src, scalar=0.0,
                                                       in1=mt[:], op0=ALU.max, op1=ALU.add)
                    else:      # v -> v_ext[:, heads, 0:64]
                        h0 = (oc - 2) * 8
                        dst = v_ext[:, h0:h0 + 8, 0:64]
                        if has_bv:
                            vb = bass.AP(tensor=bv.ap().tensor, offset=(oc - 2) * 512,
                                         ap=[[0, 128], [64, 8], [1, 64]])
                            vb_t = p1w.tile([128, 8, 64], F32, tag="vb")
                            nc.sync.dma_start(out=vb_t[:], in_=vb)
                            nc.vector.tensor_tensor(
                                out=dst, in0=ps[:].rearrange("p (h d) -> p h d", d=64),
                                in1=vb_t[:], op=ALU.add)
                        else:
                            nc.vector.tensor_copy(
                                out=dst, in_=ps[:].rearrange("p (h d) -> p h d", d=64))
                # kv accumulation: per head [64, 65] += k_h^T @ [v_h | 1]
                for h in range(H):
                    ti, hf, slot = h // 8, (h % 8) // 4, h % 4
                    nc.tensor.matmul(
                        kv_ps[ti][hf * 64:(hf + 1) * 64, slot, :],
                        lhsT=k_sb[:, h * 64:(h + 1) * 64],
                        rhs=v_ext[:, h, :],
                        start=(tt == 0), stop=(tt == NT - 1))

            # stage kv psum -> SBUF -> DRAM, then pairwise AllReduce
            kv_st = kvstagep.tile([128, 2, 4, 65], F32)
            for ti in range(2):
                nc.vector.tensor_copy(out=kv_st[:, ti], in_=kv_ps[ti][:])
                nc.sync.dma_start(out=cci[ti], in_=kv_st[:, ti])
            if no_cc:
                nc.sync.dma_start(out=cco[:], in_=cci[:])
            else:
                nc.gpsimd.collective_compute(
                    "AllReduce", ALU.add,
                    replica_groups=[[0, 1], [2, 3], [4, 5], [6, 7]],
                    ins=[cci[:]], outs=[cco[:]])

        # ------------- Phase 1b: qT (overlaps the collective) -------------
        big_cm = tc.tile_pool(name="big", bufs=4, side="right")
        big = big_cm.__enter__()
        qT_g = [big.tile([128, 8, 512], F32R, tag="grp", name=f"qT_g{i}") for i in range(NG)]
        with tc.tile_pool(name="wqp", bufs=1) as wqp, \
             tc.tile_pool(name="p1bw", bufs=3) as p1bw, \
             tc.tile_pool(name="qhtp", bufs=2) as qhtp, \
             tc.tile_pool(name="q_ps", bufs=4, space="PSUM") as qpsp:
            wq_sb = wqp.tile([128, 8, C], F32R)
            wq_v = wq.ap().rearrange("(cc p) o -> p cc o", p=128).bitcast(F32R)
            for oc in range(4):
                nc.sync.dma_start(out=wq_sb[:, :, oc * 256:(oc + 1) * 256],
                                  in_=wq_v[:, :, oc * 256:(oc + 1) * 256])
            for g in range(NG):
                qht = qhtp.tile([128, 8, 4, 128], F32R)
                nc.sync.dma_start(out=qht[:], in_=ht_d[4 * g:4 * (g + 1)].rearrange(
                    "tl p cc t -> p cc tl t").bitcast(F32R))
                qht_v = qht[:].rearrange("p cc tl t -> p cc (tl t)")
                for oc in range(8):
                    ps = qpsp.tile([128, 512], F32)
                    for cc in range(8):
                        nc.tensor.matmul(ps[:], lhsT=wq_sb[:, cc, oc * 128:(oc + 1) * 128],
                                         rhs=qht_v[:, cc, :],
                                         start=(cc == 0), stop=(cc == 7))
                    mt = p1bw.tile([128, 512], F32, tag="phim")
                    rt = p1bw.tile([128, 512], F32, tag="phir")
                    nc.vector.tensor_scalar(out=mt[:], in0=ps[:], scalar1=bq_sb[:, oc:oc + 1],
                                            scalar2=0.0, op0=ALU.add, op1=ALU.min)
                    nc.scalar.activation(out=mt[:], in_=mt[:], func=AF.Exp)
                    nc.vector.tensor_scalar(out=rt[:], in0=ps[:], scalar1=bq_sb[:, oc:oc + 1],
                                            scalar2=0.0, op0=ALU.add, op1=ALU.max)
                    nc.vector.tensor_tensor(out=mt[:], in0=mt[:], in1=rt[:], op=ALU.add)
                    nc.sync.dma_start(out=qT_g[g][:, oc, :], in_=mt[:].bitcast(F32R))

        # ---------------- Phase 2: attention + proj + LN2 ----------------
        with (
            tc.tile_pool(name="wpp", bufs=1) as wpp,
            tc.tile_pool(name="kv2", bufs=1) as kv2p,
            tc.tile_pool(name="p2w", bufs=2) as p2w,
            tc.tile_pool(name="p2w1", bufs=3) as p2w1,
            tc.tile_pool(name="attnt", bufs=1) as attntp,
            tc.tile_pool(name="zbcpa", bufs=1) as zbcpa,
            tc.tile_pool(name="z_ps", bufs=2, space="PSUM") as zpsp,
            tc.tile_pool(name="attn_ps", bufs=2, space="PSUM") as attnpsp,
            tc.tile_pool(name="proj_ps", bufs=2, space="PSUM") as projpsp,
            tc.tile_pool(name="tr2_ps", bufs=1, space="PSUM") as trps2p,
        ):
            wp_sb = wpp.tile([128, 8, C], F32R)
            nc.sync.dma_start(out=wp_sb[:], in_=wp.ap().rearrange(
                "(cc p) o -> p cc o", p=128).bitcast(F32R))
            kv_sb2 = kv2p.tile([128, 8, 65], F32R)
            kv_bd = kv2p.tile([128, 8, 128], F32R)
            nc.vector.memset(kv_bd[:].bitcast(F32), 0.0)
            bd = kv2p.tile([128, 8, 16], F32R)
            nc.vector.memset(bd[:].bitcast(F32), 0.0)
            for h in range(H):
                ti, hf, slot = h // 8, (h % 8) // 4, h % 4
                pbase = (h % 2) * 64
                nc.sync.dma_start(
                    out=kv_sb2[pbase:pbase + 64, h // 2, :],
                    in_=cco[ti, hf * 64:(hf + 1) * 64, slot, :].bitcast(F32R))
                # block-diagonal kv per head pair: head h occupies rows/cols
                # [pbase, pbase+64) of kv_bd[:, h//2, :]
                nc.sync.dma_start(
                    out=kv_bd[pbase:pbase + 64, h // 2, pbase:pbase + 64],
                    in_=kv_sb2[pbase:pbase + 64, h // 2, 0:64])
                nc.sync.dma_start(
                    out=bd[pbase:pbase + 64, h // 2, h:h + 1],
                    in_=kv_sb2[pbase:pbase + 64, h // 2, 64:65])
            z_bcs = {}

            def emit_z(g):
                # z = 1 / (q . ksum + eps), then broadcast to head-pair layout
                zps = zpsp.tile([16, 512], F32, name=f"zps{g}", tag="zps")
                for pc in range(8):
                    nc.tensor.matmul(zps[:], lhsT=bd[:, pc, :], rhs=qT_g[g][:, pc, :],
                                     start=(pc == 0), stop=(pc == 7))
                zsl = p2w.tile([16, 512], F32, name=f"zt{g}", tag="zt")
                nc.vector.tensor_scalar_add(out=zsl[:], in0=zps[:], scalar1=EPS_ATTN)
                nc.vector.reciprocal(out=zsl[:], in_=zsl[:])
                nc.sync.dma_start(out=z_d[:, g * 512:(g + 1) * 512], in_=zsl[:])
                z_bc = zbcpa.tile([128, 8, 512], F32, name=f"zbc{g}", tag="zbc")
                zd_ap = z_d[:]
                for sub in range(2):
                    nc.sync.dma_start(
                        out=z_bc[sub * 64:(sub + 1) * 64, :, :],
                        in_=bass.AP(tensor=zd_ap.tensor,
                                    offset=zd_ap.offset + sub * TOK + g * 512,
                                    ap=[[0, 64], [2 * TOK, 8], [1, 512]]))
                z_bcs[g] = z_bc

            emit_z(0)
            for g in range(NG):
                if g + 1 < NG:
                    emit_z(g + 1)
                z_bc = z_bcs.pop(g)
                # attn_T = (kv_h^T q_h) * z, head pairs share a psum bank
                attn_r = attntp.tile([128, 8, 512], F32R)
                for cc in range(8):
                    aps = attnpsp.tile([128, 512], F32)
                    nc.tensor.matmul(aps[:], lhsT=kv_bd[:, cc, :],
                                     rhs=qT_g[g][:, cc, :], start=True, stop=True)
                    attn_tmp = p2w.tile([128, 512], F32, tag="attn_tmp", name=f"attn_tmp{g}_{cc}")
                    nc.vector.tensor_tensor(out=attn_tmp[:], in0=aps[:],
                                            in1=z_bc[:, cc, :], op=ALU.mult)
                    nc.sync.dma_start(out=attn_r[:, cc, :], in_=attn_tmp[:].bitcast(F32R))

                # proj + residual -> x1; LN2; transpose -> h2T group tile
                h2T = big.tile([128, 8, 512], F32R, tag="grp", name=f"h2T_g{g}")
                for tl in range(4):
                    tt = g * 4 + tl
                    x_rel = p2w.tile([128, C], F32, tag="xrel")
                    nc.sync.dma_start(out=x_rel[:], in_=xs_v[tt])
                    x1_t = p2w.tile([128, C], F32, tag="x1")
                    for oc in range(2):
                        pps = projpsp.tile([128, 512], F32)
                        for cc in range(8):
                            nc.tensor.matmul(pps[:], lhsT=attn_r[:, cc, tl * 128:(tl + 1) * 128],
                                             rhs=wp_sb[:, cc, oc * 512:(oc + 1) * 512],
                                             start=(cc == 0), stop=(cc == 7))
                        osl = slice(oc * 512, (oc + 1) * 512)
                        nc.vector.tensor_tensor(out=x1_t[:, osl], in0=pps[:],
                                                in1=x_rel[:, osl], op=ALU.add)
                        if has_bproj:
                            nc.vector.tensor_tensor(out=x1_t[:, osl], in0=x1_t[:, osl],
                                                    in1=bp_bc[:, osl], op=ALU.add)
                    nc.sync.dma_start(out=x1s[tt], in_=x1_t[:])
                    h2_t = p2w1.tile([128, C], F32, tag="h2")
                    _emit_ln(nc, pools, x1_t, eps_ln_t, h2_t)
                    tr_ps2 = trps2p.tile([128, 8, 128], F32)
                    for cc in range(8):
                        nc.tensor.transpose(tr_ps2[:, cc, :], h2_t[:, cc * 128:(cc + 1) * 128], ident[:])
                    tr_tmp2 = p2w1.tile([128, 8, 128], F32, tag="tr2")
                    nc.vector.tensor_copy(out=tr_tmp2[:], in_=tr_ps2[:])
                    nc.sync.dma_start(out=h2T[:, :, tl * 128:(tl + 1) * 128],
                                        in_=tr_tmp2[:].bitcast(F32R))
                qT_g[g] = h2T  # slot reuse: qT_g[g] fully consumed above

        h2T_g = qT_g  # now holds h2T group tiles

        w2_v = w2.ap().rearrange("(hc p) o -> p hc o", p=128).bitcast(F32R)

        # ---------------- Phase 3a: fc1 + gelu -> h3s (DRAM) ----------------
        with tc.tile_pool(name="w1p", bufs=2) as w1p, \
             tc.tile_pool(name="gelt", bufs=2) as geltp, \
             tc.tile_pool(name="f1_ps", bufs=4, space="PSUM") as f1psp:
            w1_v = w1.ap().rearrange("(cc p) o -> p cc o", p=128)
            for hd in range(32):
                w1_col = w1p.tile([128, 8, 128], F32R)
                nc.sync.dma_start(out=w1_col[:],
                                    in_=w1_v[:, :, hd * 128:(hd + 1) * 128].bitcast(F32R))
                for g in range(NG):
                    ps = f1psp.tile([128, 512], F32)
                    for cc in range(8):
                        nc.tensor.matmul(ps[:], lhsT=w1_col[:, cc, :],
                                         rhs=h2T_g[g][:, cc, :],
                                         start=(cc == 0), stop=(cc == 7))
                    gt = geltp.tile([128, 512], F32)
                    nc.scalar.activation(out=gt[:], in_=ps[:], func=AF.Gelu,
                                         bias=bg_sb[:, hd:hd + 1], scale=1.0)
                    nc.sync.dma_start(out=h3s[hd, :, g * 512:(g + 1) * 512], in_=gt[:])

        big_cm.__exit__(None, None, None)

        # ---------------- Phase 3b: fc2 + residual -> out ----------------
        with tc.tile_pool(name="w2p", bufs=1) as w2p, \
             tc.tile_pool(name="h3c", bufs=3) as h3cp, \
             tc.tile_pool(name="outp", bufs=2) as outp, \
             tc.tile_pool(name="f2_ps", bufs=3, space="PSUM") as f2psp:
            w2_sb = w2p.tile([128, 32, C], F32R)
            for hc in range(8):
                nc.sync.dma_start(out=w2_sb[:, 4 * hc:4 * (hc + 1), :],
                                  in_=w2_v[:, 4 * hc:4 * (hc + 1), :])
            h3s_v = h3s[:].rearrange("hd p t -> p hd t")
            for tt in range(NT):
                ps = f2psp.tile([128, C], F32)
                h3c = h3cp.tile([128, 32, 128], F32R)
                nc.sync.dma_start(out=h3c[:],
                                  in_=h3s_v[:, :, tt * 128:(tt + 1) * 128].bitcast(F32R))
                for hd in range(32):
                    for oc in range(2):
                        nc.tensor.matmul(ps[:, oc * 512:(oc + 1) * 512], lhsT=h3c[:, hd, :],
                                         rhs=w2_sb[:, hd, oc * 512:(oc + 1) * 512],
                                         start=(hd == 0), stop=(hd == 31))
                x1_rel = outp.tile([128, C], F32, tag="x1rel")
                nc.sync.dma_start(out=x1_rel[:], in_=x1s[tt])
                o_t = outp.tile([128, C], F32, tag="ot")
                nc.vector.tensor_tensor(out=o_t[:], in0=ps[:], in1=x1_rel[:], op=ALU.add)
                if has_bfc2:
                    nc.vector.tensor_tensor(out=o_t[:], in0=o_t[:], in1=b2_bc[:], op=ALU.add)
                nc.sync.dma_start(out=out_v[tt], in_=o_t[:])

    nc.compile()
    return nc


def _prep_inputs(x, norm1_g, norm1_b, qkv_w, proj_w, proj_b, norm2_g, norm2_b,
                 fc1_w, fc1_b, fc2_w, fc2_b):
    """Host-side weight prep. Folds LN gains into weights; LN biases into
    per-output biases. Returns (flags, per-core in_maps)."""
    x = np.asarray(x, np.float32)
    g1 = np.asarray(norm1_g, np.float32)
    b1 = np.asarray(norm1_b, np.float32)
    qkv_w = np.asarray(qkv_w, np.float32)
    proj_w = np.asarray(proj_w, np.float32)
    proj_b = np.asarray(proj_b, np.float32)
    g2 = np.asarray(norm2_g, np.float32)
    b2 = np.asarray(norm2_b, np.float32)
    fc1_w = np.asarray(fc1_w, np.float32)
    fc1_b = np.asarray(fc1_b, np.float32)
    fc2_w = np.asarray(fc2_w, np.float32)
    fc2_b = np.asarray(fc2_b, np.float32)

    wq_t = np.ascontiguousarray((qkv_w[0:C] * g1[None, :]).T)            # [c, o]
    wkv_t = np.ascontiguousarray((qkv_w[C:3 * C] * g1[None, :]).T)       # [c, 2C]
    wp_t = np.ascontiguousarray(proj_w.T)
    w1_t = np.ascontiguousarray((fc1_w * g2[None, :]).T)                 # [c, HID]
    w2_t = np.ascontiguousarray(fc2_w.T)                                 # [HID, c]
    bq_v = qkv_w[0:C] @ b1
    bk_v = qkv_w[C:2 * C] @ b1
    bv_v = qkv_w[2 * C:3 * C] @ b1
    bg_v = fc1_w @ b2 + fc1_b

    flags = (bool(np.any(bk_v)), bool(np.any(bv_v)),
             bool(np.any(proj_b)), bool(np.any(fc2_b)))

    shared = dict(wq=wq_t, wkv=wkv_t, wp=wp_t, w1=w1_t, w2=w2_t,
                  bq=np.ascontiguousarray(bq_v, dtype=np.float32),
                  bk=np.ascontiguousarray(bk_v, dtype=np.float32),
                  bv=np.ascontiguousarray(bv_v, dtype=np.float32),
                  bg=np.ascontiguousarray(bg_v, dtype=np.float32),
                  bp=proj_b, b2o=fc2_b)
    in_maps = []
    for core in range(8):
        b, half = core // 2, core % 2
        xs = np.ascontiguousarray(x[b, half * TOK:(half + 1) * TOK, :])
        in_maps.append({"xs": xs, **shared})
    return flags, in_maps


def get_compiled(flags):
    if flags not in _BUILD_CACHE:
        _BUILD_CACHE[flags] = _build(flags)
    return _BUILD_CACHE[flags]


def kernel(**inputs) -> np.ndarray:
    flags, in_maps = _prep_inputs(**inputs)
    nc = get_compiled(flags)
    res = run_bass_kernel_spmd(nc, in_maps=in_maps, core_ids=list(range(8)))
    shards = [res.results[c]["out"] for c in range(8)]
    full = np.empty((B, N, C), np.float32)
    for core in range(8):
        b, half = core // 2, core % 2
        full[b, half * TOK:(half + 1) * TOK, :] = shards[core]
    return full


# revision 29
# speedup vs baseline: 1.1023x; 1.0576x over previous
"""Trainium2 Bass kernel for nn_Block_9457517985872 (dense transformer block
with linear attention). Token-sharded across 8 NeuronCores: core c handles
batch c//2, sequence half c%2 (2048 tokens). Only cross-core communication is
a pairwise AllReduce of the per-head (kv, ksum) statistics [16,64,65] f32.

Self-contained: hardcodes all shapes from the problem spec.
"""
import numpy as np
from contextlib import ExitStack

import concourse.bass as bass
import concourse.tile as tile
from concourse import bacc, mybir
from concourse.bass_utils import run_bass_kernel_spmd
from concourse.masks import make_identity

F32 = mybir.dt.float32
F32R = mybir.dt.float32r
AF = mybir.ActivationFunctionType
ALU = mybir.AluOpType

B, N, C = 4, 4096, 1024
H, D = 16, 64
HID = 4096
TOK = 2048          # tokens per core
NT = TOK // 128     # 16 token tiles
NG = TOK // 512     # 4 token groups
EPS_LN = 1e-5
EPS_ATTN = 1e-6

_BUILD_CACHE = {}


def _emit_ln(nc, pools, x_t, eps_t, out_t):
    """LayerNorm core (no gamma/beta): out = (x - mean(x)) * rsqrt(var + eps).
    x_t: [128, 1024] f32 SBUF tile. out_t may alias x_t."""
    stats = pools["stat"].tile([128, 2, 6], F32, name="ln_stats", tag="ln_stats")
    mv = pools["stat"].tile([128, 2], F32, name="ln_mv", tag="ln_mv")
    for sg in range(2):
        nc.vector.bn_stats(out=stats[:, sg, :], in_=x_t[:, sg * 512:(sg + 1) * 512])
    nc.vector.bn_aggr(out=mv[:], in_=stats[:])
    # mv[:,0]=mean, mv[:,1]=var -> rstd
    nc.scalar.activation(out=mv[:, 1:2], in_=mv[:, 1:2], func=AF.Sqrt, bias=eps_t[:], scale=1.0)
    nc.vector.reciprocal(out=mv[:, 1:2], in_=mv[:, 1:2])
    # mv[:,0] = -mean*rstd
    nc.vector.tensor_tensor(out=mv[:, 0:1], in0=mv[:, 0:1], in1=mv[:, 1:2], op=ALU.mult)
    nc.vector.tensor_scalar_mul(out=mv[:, 0:1], in0=mv[:, 0:1], scalar1=-1.0)
    nc.scalar.activation(out=out_t[:], in_=x_t[:], func=AF.Identity,
                         bias=mv[:, 0:1], scale=mv[:, 1:2])


def _build(flags, no_cc=False):
    """flags: (has_bk, has_bv, has_bproj, has_bfc2)"""
    has_bk, has_bv, has_bproj, has_bfc2 = flags
    nc = bacc.Bacc("TRN2", target_bir_lowering=False, debug=False,
                   num_devices=1 if no_cc else 8)

    xs = nc.dram_tensor("xs", [TOK, C], F32, kind="ExternalInput")
    wq = nc.dram_tensor("wq", [C, C], F32, kind="ExternalInput")      # [c, o]
    wkv = nc.dram_tensor("wkv", [C, 2 * C], F32, kind="ExternalInput")
    wp = nc.dram_tensor("wp", [C, C], F32, kind="ExternalInput")
    w1 = nc.dram_tensor("w1", [C, HID], F32, kind="ExternalInput")
    w2 = nc.dram_tensor("w2", [HID, C], F32, kind="ExternalInput")
    bq = nc.dram_tensor("bq", [C], F32, kind="ExternalInput")
    bk = nc.dram_tensor("bk", [C], F32, kind="ExternalInput")
    bv = nc.dram_tensor("bv", [C], F32, kind="ExternalInput")
    bg = nc.dram_tensor("bg", [HID], F32, kind="ExternalInput")
    bp = nc.dram_tensor("bp", [C], F32, kind="ExternalInput")
    b2o = nc.dram_tensor("b2o", [C], F32, kind="ExternalInput")
    out = nc.dram_tensor("out", [TOK, C], F32, kind="ExternalOutput")

    xs_v = xs.ap().rearrange("(t p) c -> t p c", p=128)     # [16,128,1024]
    out_v = out.ap().rearrange("(t p) c -> t p c", p=128)

    with tile.TileContext(nc) as tc, ExitStack() as ctx:
        const = ctx.enter_context(tc.tile_pool(name="const", bufs=1))
        dram = ctx.enter_context(tc.tile_pool(name="dram", bufs=1, space="DRAM"))
        statp = ctx.enter_context(tc.tile_pool(name="stat", bufs=4))
        pools = {"stat": statp}

        ident = const.tile([128, 128], F32)
        make_identity(nc, ident[:])
        eps_ln_t = const.tile([128, 1], F32)
        nc.vector.memset(eps_ln_t[:], EPS_LN)
        bq_sb = const.tile([128, 8], F32)
        nc.sync.dma_start(out=bq_sb[:], in_=bq.ap().rearrange("(oc p) -> p oc", p=128))
        bg_sb = const.tile([128, 32], F32)
        nc.sync.dma_start(out=bg_sb[:], in_=bg.ap().rearrange("(hd p) -> p hd", p=128))
        if has_bk:
            bk_bc = const.tile([128, C], F32)
            nc.sync.dma_start(out=bk_bc[:], in_=bass.AP(
                tensor=bk.ap().tensor, offset=0, ap=[[0, 128], [1, C]]))
        if has_bproj:
            bp_bc = const.tile([128, C], F32)
            nc.sync.dma_start(out=bp_bc[:], in_=bass.AP(
                tensor=bp.ap().tensor, offset=0, ap=[[0, 128], [1, C]]))
        if has_bfc2:
            b2_bc = const.tile([128, C], F32)
            nc.sync.dma_start(out=b2_bc[:], in_=bass.AP(
                tensor=b2o.ap().tensor, offset=0, ap=[[0, 128], [1, C]]))

        x1s = dram.tile([NT, 128, C], F32)
        h3s = dram.tile([32, 128, TOK], F32)
        cci = dram.tile([2, 128, 4, 65], F32)
        cco = dram.tile([2, 128, 4, 65], F32)
        z_d = dram.tile([16, TOK], F32)
        ht_d = dram.tile([NT, 128, 8, 128], F32)

        # ---------------- Phase 1a: LN1, hT, k/v, kv+ksum ----------------
        with (
            tc.tile_pool(name="wkvp", bufs=1) as wkvp,
            tc.tile_pool(name="p1w", bufs=3) as p1w,
            tc.tile_pool(name="htrp", bufs=2) as htrp,
            tc.tile_pool(name="trtmp", bufs=2) as trtmpp,
            tc.tile_pool(name="kvstage", bufs=1) as kvstagep,
            tc.tile_pool(name="kvacc_ps", bufs=1, space="PSUM") as kvaccp,
            tc.tile_pool(name="tr_ps", bufs=1, space="PSUM") as trpsp,
            tc.tile_pool(name="gen_ps", bufs=4, space="PSUM") as genpsp,
        ):
            x_tiles = {}
            for tt in range(2):
                x_t = p1w.tile([128, C], F32, tag="x", name=f"x_t{tt}")
                nc.sync.dma_start(out=x_t[:], in_=xs_v[tt])
                x_tiles[tt] = x_t
            wkv_sb = wkvp.tile([128, 8, 2 * C], F32R)
            wkv_v = wkv.ap().rearrange("(cc p) o -> p cc o", p=128).bitcast(F32R)
            for oc in range(4):
                nc.sync.dma_start(out=wkv_sb[:, :, oc * 512:(oc + 1) * 512],
                                  in_=wkv_v[:, :, oc * 512:(oc + 1) * 512])
            kv_ps = [kvaccp.tile([128, 4, 65], F32, name=f"kv_ps{i}") for i in range(2)]

            for tt in range(NT):
                if tt in x_tiles:
                    x_t = x_tiles.pop(tt)
                else:
                    x_t = p1w.tile([128, C], F32, tag="x", name=f"x_t{tt}")
                    nc.sync.dma_start(out=x_t[:], in_=xs_v[tt])
                _emit_ln(nc, pools, x_t, eps_ln_t, x_t)
                # transpose h (=x_t) -> hT_full[:, :, tt*128:+128]
                tr_ps = trpsp.tile([128, 8, 128], F32)
                for cc in range(8):
                    nc.tensor.transpose(tr_ps[:, cc, :], x_t[:, cc * 128:(cc + 1) * 128], ident[:])
                tr_tmp = trtmpp.tile([128, 8, 128], F32)
                nc.vector.tensor_copy(out=tr_tmp[:], in_=tr_ps[:])
                nc.sync.dma_start(out=ht_d[tt], in_=tr_tmp[:])
                hT_r = htrp.tile([128, 8, 128], F32R)
                nc.sync.dma_start(out=hT_r[:], in_=tr_tmp[:].bitcast(F32R))
                # k, v for this tile
                k_sb = p1w.tile([128, C], F32, tag="k")
                v_ext = p1w.tile([128, H, 65], F32, tag="v")
                nc.vector.memset(v_ext[:, :, 64:65], 1.0)
                for oc in range(4):
                    ps = genpsp.tile([128, 512], F32, tag="gen")
                    for cc in range(8):
                        nc.tensor.matmul(ps[:], lhsT=hT_r[:, cc, :],
                                         rhs=wkv_sb[:, cc, oc * 512:(oc + 1) * 512],
                                         start=(cc == 0), stop=(cc == 7))
                    if oc < 2:  # k: phi = exp(min(x,0)) + relu(x)
                        ksl = k_sb[:, oc * 512:(oc + 1) * 512]
                        if has_bk:
                            nc.vector.tensor_tensor(out=ksl, in0=ps[:],
                                                    in1=bk_bc[:, oc * 512:(oc + 1) * 512], op=ALU.add)
                            src = ksl
                        else:
                            src = ps[:]
                        mt = p1w.tile([128, 512], F32, tag="phim")
                        nc.vector.tensor_scalar_min(out=mt[:], in0=src, scalar1=0.0)
                        nc.scalar.activation(out=mt[:], in_=mt[:], func=AF.Exp)
                        nc.vector.scalar_tensor_tensor(out=ksl, in0=src, scalar=0.0,
                                                       in1=mt[:], op0=ALU.max, op1=ALU.add)
                    else:      # v -> v_ext[:, heads, 0:64]
                        h0 = (oc - 2) * 8
                        dst = v_ext[:, h0:h0 + 8, 0:64]
                        if has_bv:
                            vb = bass.AP(tensor=bv.ap().tensor, offset=(oc - 2) * 512,
                                         ap=[[0, 128], [64, 8], [1, 64]])
                            vb_t = p1w.tile([128, 8, 64], F32, tag="vb")
                            nc.sync.dma_start(out=vb_t[:], in_=vb)
                            nc.vector.tensor_tensor(
                                out=dst, in0=ps[:].rearrange("p (h d) -> p h d", d=64),
                                in1=vb_t[:], op=ALU.add)
                        else:
                            nc.vector.tensor_copy(
                                out=dst, in_=ps[:].rearrange("p (h d) -> p h d", d=64))
                # kv accumulation: per head [64, 65] += k_h^T @ [v_h | 1]
                for h in range(H):
                    ti, hf, slot = h // 8, (h % 8) // 4, h % 4
                    nc.tensor.matmul(
                        kv_ps[ti][hf * 64:(hf + 1) * 64, slot, :],
                        lhsT=k_sb[:, h * 64:(h + 1) * 64],
                        rhs=v_ext[:, h, :],
                        start=(tt == 0), stop=(tt == NT - 1))

            # stage kv psum -> SBUF -> DRAM, then pairwise AllReduce
            kv_st = kvstagep.tile([128, 2, 4, 65], F32)
            for ti in range(2):
                nc.vector.tensor_copy(out=kv_st[:, ti], in_=kv_ps[ti][:])
                nc.sync.dma_start(out=cci[ti], in_=kv_st[:, ti])
            if no_cc:
                nc.sync.dma_start(out=cco[:], in_=cci[:])
            else:
                nc.gpsimd.collective_compute(
                    "AllReduce", ALU.add,
                    replica_groups=[[0, 1], [2, 3], [4, 5], [6, 7]],
                    ins=[cci[:]], outs=[cco[:]])

        # ------------- Phase 1b: qT (overlaps the collective) -------------
        big_cm = tc.tile_pool(name="big", bufs=4, side="right")
        big = big_cm.__enter__()
        qT_g = [big.tile([128, 8, 512], F32R, tag="grp", name=f"qT_g{i}") for i in range(NG)]
        with tc.tile_pool(name="wqp", bufs=1) as wqp, \
             tc.tile_pool(name="p1bw", bufs=3) as p1bw, \
             tc.tile_pool(name="qhtp", bufs=2) as qhtp, \
             tc.tile_pool(name="q_ps", bufs=4, space="PSUM") as qpsp:
            wq_sb = wqp.tile([128, 8, C], F32R)
            wq_v = wq.ap().rearrange("(cc p) o -> p cc o", p=128).bitcast(F32R)
            for oc in range(4):
                nc.sync.dma_start(out=wq_sb[:, :, oc * 256:(oc + 1) * 256],
                                  in_=wq_v[:, :, oc * 256:(oc + 1) * 256])
            for g in range(NG):
                qht = qhtp.tile([128, 8, 4, 128], F32R)
                nc.sync.dma_start(out=qht[:], in_=ht_d[4 * g:4 * (g + 1)].rearrange(
                    "tl p cc t -> p cc tl t").bitcast(F32R))
                qht_v = qht[:].rearrange("p cc tl t -> p cc (tl t)")
                for oc in range(8):
                    ps = qpsp.tile([128, 512], F32)
                    for cc in range(8):
                        nc.tensor.matmul(ps[:], lhsT=wq_sb[:, cc, oc * 128:(oc + 1) * 128],
                                         rhs=qht_v[:, cc, :],
                                         start=(cc == 0), stop=(cc == 7))
                    mt = p1bw.tile([128, 512], F32, tag="phim")
                    rt = p1bw.tile([128, 512], F32, tag="phir")
                    nc.vector.tensor_scalar(out=mt[:], in0=ps[:], scalar1=bq_sb[:, oc:oc + 1],
                                            scalar2=0.0, op0=ALU.add, op1=ALU.min)
                    nc.scalar.activation(out=mt[:], in_=mt[:], func=AF.Exp)
                    nc.vector.tensor_scalar(out=rt[:], in0=ps[:], scalar1=bq_sb[:, oc:oc + 1],
                                            scalar2=0.0, op0=ALU.add, op1=ALU.max)
                    nc.vector.tensor_tensor(out=mt[:], in0=mt[:], in1=rt[:], op=ALU.add)
                    nc.sync.dma_start(out=qT_g[g][:, oc, :], in_=mt[:].bitcast(F32R))

        # ---------------- Phase 2: attention + proj + LN2 ----------------
        with (
            tc.tile_pool(name="wpp", bufs=1) as wpp,
            tc.tile_pool(name="kv2", bufs=1) as kv2p,
            tc.tile_pool(name="p2w", bufs=2) as p2w,
            tc.tile_pool(name="p2w1", bufs=3) as p2w1,
            tc.tile_pool(name="attnt", bufs=1) as attntp,
            tc.tile_pool(name="zbcpa", bufs=1) as zbcpa,
            tc.tile_pool(name="z_ps", bufs=2, space="PSUM") as zpsp,
            tc.tile_pool(name="attn_ps", bufs=2, space="PSUM") as attnpsp,
            tc.tile_pool(name="proj_ps", bufs=2, space="PSUM") as projpsp,
            tc.tile_pool(name="tr2_ps", bufs=1, space="PSUM") as trps2p,
        ):
            wp_sb = wpp.tile([128, 8, C], F32R)
            nc.sync.dma_start(out=wp_sb[:], in_=wp.ap().rearrange(
                "(cc p) o -> p cc o", p=128).bitcast(F32R))
            kv_sb2 = kv2p.tile([128, 8, 65], F32R)
            kv_bd = kv2p.tile([128, 8, 128], F32R)
            nc.vector.memset(kv_bd[:].bitcast(F32), 0.0)
            bd = kv2p.tile([128, 8, 16], F32R)
            nc.vector.memset(bd[:].bitcast(F32), 0.0)
            for h in range(H):
                ti, hf, slot = h // 8, (h % 8) // 4, h % 4
                pbase = (h % 2) * 64
                nc.sync.dma_start(
                    out=kv_sb2[pbase:pbase + 64, h // 2, :],
                    in_=cco[ti, hf * 64:(hf + 1) * 64, slot, :].bitcast(F32R))
                # block-diagonal kv per head pair: head h occupies rows/cols
                # [pbase, pbase+64) of kv_bd[:, h//2, :]
                nc.sync.dma_start(
                    out=kv_bd[pbase:pbase + 64, h // 2, pbase:pbase + 64],
                    in_=kv_sb2[pbase:pbase + 64, h // 2, 0:64])
                nc.sync.dma_start(
                    out=bd[pbase:pbase + 64, h // 2, h:h + 1],
                    in_=kv_sb2[pbase:pbase + 64, h // 2, 64:65])
            z_bcs = {}

            def emit_z(g):
                # z = 1 / (q . ksum + eps), then broadcast to head-pair layout
                zps = zpsp.tile([16, 512], F32, name=f"zps{g}", tag="zps")
                for pc in range(8):
                    nc.tensor.matmul(zps[:], lhsT=bd[:, pc, :], rhs=qT_g[g][:, pc, :],
                                     start=(pc == 0), stop=(pc == 7))
                zsl = p2w.tile([16, 512], F32, name=f"zt{g}", tag="zt")
                nc.vector.tensor_scalar_add(out=zsl[:], in0=zps[:], scalar1=EPS_ATTN)
                nc.vector.reciprocal(out=zsl[:], in_=zsl[:])
                nc.sync.dma_start(out=z_d[:, g * 512:(g + 1) * 512], in_=zsl[:])
                z_bc = zbcpa.tile([128, 8, 512], F32, name=f"zbc{g}", tag="zbc")
                zd_ap = z_d[:]
                for sub in range(2):
                    nc.sync.dma_start(
                        out=z_bc[sub * 64:(sub + 1) * 64, :, :],
                        in_=bass.AP(tensor=zd_ap.tensor,
                                    offset=zd_ap.offset + sub * TOK + g * 512,
                                    ap=[[0, 64], [2 * TOK, 8], [1, 512]]))
                z_bcs[g] = z_bc

            emit_z(0)
            for g in range(NG):
                if g + 1 < NG:
                    emit_z(g + 1)
                z_bc = z_bcs.pop(g)
                # attn_T = (kv_h^T q_h) * z, head pairs share a psum bank
                attn_r = attntp.tile([128, 8, 512], F32R)
                for cc in range(8):
                    aps = attnpsp.tile([128, 512], F32)
                    nc.tensor.matmul(aps[:], lhsT=kv_bd[:, cc, :],
                                     rhs=qT_g[g][:, cc, :], start=True, stop=True)
                    attn_tmp = p2w.tile([128, 512], F32, tag="attn_tmp", name=f"attn_tmp{g}_{cc}")
                    nc.vector.tensor_tensor(out=attn_tmp[:], in0=aps[:],
                                            in1=z_bc[:, cc, :], op=ALU.mult)
                    nc.sync.dma_start(out=attn_r[:, cc, :], in_=attn_tmp[:].bitcast(F32R))

                # proj + residual -> x1; LN2; transpose -> h2T group tile
                h2T = big.tile([128, 8, 512], F32R, tag="grp", name=f"h2T_g{g}")
                for tl in range(4):
                    tt = g * 4 + tl
                    x_rel = p2w.tile([128, C], F32, tag="xrel")
                    nc.sync.dma_start(out=x_rel[:], in_=xs_v[tt])
                    x1_t = p2w.tile([128, C], F32, tag="x1")
                    for oc in range(2):
                        pps = projpsp.tile([128, 512], F32)
                        for cc in range(8):
                            nc.tensor.matmul(pps[:], lhsT=attn_r[:, cc, tl * 128:(tl + 1) * 128],
                                             rhs=wp_sb[:, cc, oc * 512:(oc + 1) * 512],
                                             start=(cc == 0), stop=(cc == 7))
                        osl = slice(oc * 512, (oc + 1) * 512)
                        nc.vector.tensor_tensor(out=x1_t[:, osl], in0=pps[:],
                                                in1=x_rel[:, osl], op=ALU.add)
                        if has_bproj:
                            nc.vector.tensor_tensor(out=x1_t[:, osl], in0=x1_t[:, osl],
                                                    in1=bp_bc[:, osl], op=ALU.add)
                    nc.sync.dma_start(out=x1s[tt], in_=x1_t[:])
                    h2_t = p2w1.tile([128, C], F32, tag="h2")
                    _emit_ln(nc, pools, x1_t, eps_ln_t, h2_t)
                    tr_ps2 = trps2p.tile([128, 8, 128], F32)
                    for cc in range(8):
                        nc.tensor.transpose(tr_ps2[:, cc, :], h2_t[:, cc * 128:(cc + 1) * 128], ident[:])
                    tr_tmp2 = p2w1.tile([128, 8, 128], F32, tag="tr2")
                    nc.vector.tensor_copy(out=tr_tmp2[:], in_=tr_ps2[:])
                    nc.sync.dma_start(out=h2T[:, :, tl * 128:(tl + 1) * 128],
                                        in_=tr_tmp2[:].bitcast(F32R))
                qT_g[g] = h2T  # slot reuse: qT_g[g] fully consumed above

        h2T_g = qT_g  # now holds h2T group tiles

        w2_v = w2.ap().rearrange("(hc p) o -> p hc o", p=128).bitcast(F32R)

        # ---------------- Phase 3a: fc1 + gelu -> h3s (DRAM) ----------------
        with tc.tile_pool(name="w1p", bufs=2) as w1p, \
             tc.tile_pool(name="gelt", bufs=2) as geltp, \
             tc.tile_pool(name="f1_ps", bufs=4, space="PSUM") as f1psp:
            w1_v = w1.ap().rearrange("(cc p) o -> p cc o", p=128)
            for hd in range(32):
                w1_col = w1p.tile([128, 8, 128], F32R)
                nc.sync.dma_start(out=w1_col[:],
                                    in_=w1_v[:, :, hd * 128:(hd + 1) * 128].bitcast(F32R))
                for g in range(NG):
                    ps = f1psp.tile([128, 512], F32)
                    for cc in range(8):
                        nc.tensor.matmul(ps[:], lhsT=w1_col[:, cc, :],
                                         rhs=h2T_g[g][:, cc, :],
                                         start=(cc == 0), stop=(cc == 7))
                    gt = geltp.tile([128, 512], F32)
                    nc.scalar.activation(out=gt[:], in_=ps[:], func=AF.Gelu,
                                         bias=bg_sb[:, hd:hd + 1], scale=1.0)
                    nc.sync.dma_start(out=h3s[hd, :, g * 512:(g + 1) * 512], in_=gt[:])

        big_cm.__exit__(None, None, None)

        # ---------------- Phase 3b: fc2 + residual -> out ----------------
        with tc.tile_pool(name="w2p", bufs=1) as w2p, \
             tc.tile_pool(name="h3c", bufs=3) as h3cp, \
             tc.tile_pool(name="outp", bufs=2) as outp, \
             tc.tile_pool(name="f2_ps", bufs=3, space="PSUM") as f2psp:
            w2_sb = w2p.tile([128, 32, C], F32R)
            h3s_v = h3s[:].rearrange("hd p t -> p hd t")
            h3c_pre = {}
            nc.sync.dma_start(out=w2_sb[:, 0:4, :], in_=w2_v[:, 0:4, :])
            for tt in range(2):
                h3c = h3cp.tile([128, 32, 128], F32R, name=f"h3c{tt}", tag="h3c")
                nc.sync.dma_start(out=h3c[:],
                                  in_=h3s_v[:, :, tt * 128:(tt + 1) * 128].bitcast(F32R))
                h3c_pre[tt] = h3c
            for hc in range(1, 8):
                nc.sync.dma_start(out=w2_sb[:, 4 * hc:4 * (hc + 1), :],
                                  in_=w2_v[:, 4 * hc:4 * (hc + 1), :])
            for tt in range(NT):
                ps = f2psp.tile([128, C], F32)
                if tt in h3c_pre:
                    h3c = h3c_pre.pop(tt)
                else:
                    h3c = h3cp.tile([128, 32, 128], F32R, name=f"h3c{tt}", tag="h3c")
                    nc.sync.dma_start(out=h3c[:],
                                      in_=h3s_v[:, :, tt * 128:(tt + 1) * 128].bitcast(F32R))
                for hd in range(32):
                    for oc in range(2):
                        nc.tensor.matmul(ps[:, oc * 512:(oc + 1) * 512], lhsT=h3c[:, hd, :],
                                         rhs=w2_sb[:, hd, oc * 512:(oc + 1) * 512],
                                         start=(hd == 0), stop=(hd == 31))
                x1_rel = outp.tile([128, C], F32, tag="x1rel")
                nc.sync.dma_start(out=x1_rel[:], in_=x1s[tt])
                o_t = outp.tile([128, C], F32, tag="ot")
                nc.vector.tensor_tensor(out=o_t[:], in0=ps[:], in1=x1_rel[:], op=ALU.add)
                if has_bfc2:
                    nc.vector.tensor_tensor(out=o_t[:], in0=o_t[:], in1=b2_bc[:], op=ALU.add)
                nc.sync.dma_start(out=out_v[tt], in_=o_t[:])

    nc.compile()
    return nc


def _prep_inputs(x, norm1_g, norm1_b, qkv_w, proj_w, proj_b, norm2_g, norm2_b,
                 fc1_w, fc1_b, fc2_w, fc2_b):
    """Host-side weight prep. Folds LN gains into weights; LN biases into
    per-output biases. Returns (flags, per-core in_maps)."""
    x = np.asarray(x, np.float32)
    g1 = np.asarray(norm1_g, np.float32)
    b1 = np.asarray(norm1_b, np.float32)
    qkv_w = np.asarray(qkv_w, np.float32)
    proj_w = np.asarray(proj_w, np.float32)
    proj_b = np.asarray(proj_b, np.float32)
    g2 = np.asarray(norm2_g, np.float32)
    b2 = np.asarray(norm2_b, np.float32)
    fc1_w = np.asarray(fc1_w, np.float32)
    fc1_b = np.asarray(fc1_b, np.float32)
    fc2_w = np.asarray(fc2_w, np.float32)
    fc2_b = np.asarray(fc2_b, np.float32)

    wq_t = np.ascontiguousarray((qkv_w[0:C] * g1[None, :]).T)            # [c, o]
    wkv_t = np.ascontiguousarray((qkv_w[C:3 * C] * g1[None, :]).T)       # [c, 2C]
    wp_t = np.ascontiguousarray(proj_w.T)
    w1_t = np.ascontiguousarray((fc1_w * g2[None, :]).T)                 # [c, HID]
    w2_t = np.ascontiguousarray(fc2_w.T)                                 # [HID, c]
    bq_v = qkv_w[0:C] @ b1
    bk_v = qkv_w[C:2 * C] @ b1
    bv_v = qkv_w[2 * C:3 * C] @ b1
    bg_v = fc1_w @ b2 + fc1_b

    flags = (bool(np.any(bk_v)), bool(np.any(bv_v)),
             bool(np.any(proj_b)), bool(np.any(fc2_b)))

    shared = dict(wq=wq_t, wkv=wkv_t, wp=wp_t, w1=w1_t, w2=w2_t,
                  bq=np.ascontiguousarray(bq_v, dtype=np.float32),
                  bk=np.ascontiguousarray(bk_v, dtype=np.float32),
                  bv=np.ascontiguousarray(bv_v, dtype=np.float32),
                  bg=np.ascontiguousarray(bg_v, dtype=np.float32),
                  bp=proj_b, b2o=fc2_b)
    in_maps = []
    for core in range(8):
        b, half = core // 2, core % 2
        xs = np.ascontiguousarray(x[b, half * TOK:(half + 1) * TOK, :])
        in_maps.append({"xs": xs, **shared})
    return flags, in_maps


def get_compiled(flags):
    if flags not in _BUILD_CACHE:
        _BUILD_CACHE[flags] = _build(flags)
    return _BUILD_CACHE[flags]


def kernel(**inputs) -> np.ndarray:
    flags, in_maps = _prep_inputs(**inputs)
    nc = get_compiled(flags)
    res = run_bass_kernel_spmd(nc, in_maps=in_maps, core_ids=list(range(8)))
    shards = [res.results[c]["out"] for c in range(8)]
    full = np.empty((B, N, C), np.float32)
    for core in range(8):
        b, half = core // 2, core % 2
        full[b, half * TOK:(half + 1) * TOK, :] = shards[core]
    return full


# revision 31
# speedup vs baseline: 1.1720x; 1.0632x over previous
"""Trainium2 Bass kernel for nn_Block_9457517985872 (dense transformer block
with linear attention). Token-sharded across 8 NeuronCores: core c handles
batch c//2, sequence half c%2 (2048 tokens). Only cross-core communication is
a pairwise AllReduce of the per-head (kv, ksum) statistics [16,64,65] f32.

Self-contained: hardcodes all shapes from the problem spec.
"""
import numpy as np
from contextlib import ExitStack

import concourse.bass as bass
import concourse.tile as tile
from concourse import bacc, mybir
from concourse.bass_utils import run_bass_kernel_spmd
from concourse.masks import make_identity

F32 = mybir.dt.float32
F32R = mybir.dt.float32r
AF = mybir.ActivationFunctionType
ALU = mybir.AluOpType

B, N, C = 4, 4096, 1024
H, D = 16, 64
HID = 4096
TOK = 2048          # tokens per core
NT = TOK // 128     # 16 token tiles
NG = TOK // 512     # 4 token groups
EPS_LN = 1e-5
EPS_ATTN = 1e-6

_BUILD_CACHE = {}


def _emit_ln(nc, pools, x_t, eps_t, out_t):
    """LayerNorm core (no gamma/beta): out = (x - mean(x)) * rsqrt(var + eps).
    x_t: [128, 1024] f32 SBUF tile. out_t may alias x_t."""
    stats = pools["stat"].tile([128, 2, 6], F32, name="ln_stats", tag="ln_stats")
    mv = pools["stat"].tile([128, 2], F32, name="ln_mv", tag="ln_mv")
    for sg in range(2):
        nc.vector.bn_stats(out=stats[:, sg, :], in_=x_t[:, sg * 512:(sg + 1) * 512])
    nc.vector.bn_aggr(out=mv[:], in_=stats[:])
    # mv[:,0]=mean, mv[:,1]=var -> rstd
    nc.scalar.activation(out=mv[:, 1:2], in_=mv[:, 1:2], func=AF.Sqrt, bias=eps_t[:], scale=1.0)
    nc.vector.reciprocal(out=mv[:, 1:2], in_=mv[:, 1:2])
    # mv[:,0] = -mean*rstd
    nc.vector.tensor_tensor(out=mv[:, 0:1], in0=mv[:, 0:1], in1=mv[:, 1:2], op=ALU.mult)
    nc.vector.tensor_scalar_mul(out=mv[:, 0:1], in0=mv[:, 0:1], scalar1=-1.0)
    nc.scalar.activation(out=out_t[:], in_=x_t[:], func=AF.Identity,
                         bias=mv[:, 0:1], scale=mv[:, 1:2])


def _build(flags, no_cc=False):
    """flags: (has_bk, has_bv, has_bproj, has_bfc2)"""
    has_bk, has_bv, has_bproj, has_bfc2 = flags
    nc = bacc.Bacc("TRN2", target_bir_lowering=False, debug=False,
                   num_devices=1 if no_cc else 8)

    xs = nc.dram_tensor("xs", [TOK, C], F32, kind="ExternalInput")
    wq = nc.dram_tensor("wq", [C, C], F32, kind="ExternalInput")      # [c, o]
    wkv = nc.dram_tensor("wkv", [C, 2 * C], F32, kind="ExternalInput")
    wp = nc.dram_tensor("wp", [C, C], F32, kind="ExternalInput")
    w1 = nc.dram_tensor("w1", [C, HID], F32, kind="ExternalInput")
    w2 = nc.dram_tensor("w2", [HID, C], F32, kind="ExternalInput")
    bq = nc.dram_tensor("bq", [C], F32, kind="ExternalInput")
    bk = nc.dram_tensor("bk", [C], F32, kind="ExternalInput")
    bv = nc.dram_tensor("bv", [C], F32, kind="ExternalInput")
    bg = nc.dram_tensor("bg", [HID], F32, kind="ExternalInput")
    bp = nc.dram_tensor("bp", [C], F32, kind="ExternalInput")
    b2o = nc.dram_tensor("b2o", [C], F32, kind="ExternalInput")
    out = nc.dram_tensor("out", [TOK, C], F32, kind="ExternalOutput")

    xs_v = xs.ap().rearrange("(t p) c -> t p c", p=128)     # [16,128,1024]
    out_v = out.ap().rearrange("(t p) c -> t p c", p=128)

    with tile.TileContext(nc) as tc, ExitStack() as ctx:
        const = ctx.enter_context(tc.tile_pool(name="const", bufs=1))
        dram = ctx.enter_context(tc.tile_pool(name="dram", bufs=1, space="DRAM"))
        statp = ctx.enter_context(tc.tile_pool(name="stat", bufs=4))
        pools = {"stat": statp}

        ident = const.tile([128, 128], F32)
        make_identity(nc, ident[:])
        eps_ln_t = const.tile([128, 1], F32)
        nc.vector.memset(eps_ln_t[:], EPS_LN)
        bq_sb = const.tile([128, 8], F32)
        nc.sync.dma_start(out=bq_sb[:], in_=bq.ap().rearrange("(oc p) -> p oc", p=128))
        bg_sb = const.tile([128, 32], F32)
        nc.sync.dma_start(out=bg_sb[:], in_=bg.ap().rearrange("(hd p) -> p hd", p=128))
        if has_bk:
            bk_bc = const.tile([128, C], F32)
            nc.sync.dma_start(out=bk_bc[:], in_=bass.AP(
                tensor=bk.ap().tensor, offset=0, ap=[[0, 128], [1, C]]))
        if has_bproj:
            bp_bc = const.tile([128, C], F32)
            nc.sync.dma_start(out=bp_bc[:], in_=bass.AP(
                tensor=bp.ap().tensor, offset=0, ap=[[0, 128], [1, C]]))
        if has_bfc2:
            b2_bc = const.tile([128, C], F32)
            nc.sync.dma_start(out=b2_bc[:], in_=bass.AP(
                tensor=b2o.ap().tensor, offset=0, ap=[[0, 128], [1, C]]))

        x1s = dram.tile([NT, 128, C], F32)
        h3s = dram.tile([32, 128, TOK], F32)
        cci = dram.tile([2, 128, 4, 65], F32)
        cco = dram.tile([2, 128, 4, 65], F32)
        z_d = dram.tile([16, TOK], F32)
        ht_d = dram.tile([NT, 128, 8, 128], F32)

        # ---------------- Phase 1a: LN1, hT, k/v, kv+ksum ----------------
        with (
            tc.tile_pool(name="wkvp", bufs=1) as wkvp,
            tc.tile_pool(name="p1w", bufs=3) as p1w,
            tc.tile_pool(name="htrp", bufs=2) as htrp,
            tc.tile_pool(name="trtmp", bufs=2) as trtmpp,
            tc.tile_pool(name="kvstage", bufs=1) as kvstagep,
            tc.tile_pool(name="kvacc_ps", bufs=1, space="PSUM") as kvaccp,
            tc.tile_pool(name="tr_ps", bufs=1, space="PSUM") as trpsp,
            tc.tile_pool(name="gen_ps", bufs=4, space="PSUM") as genpsp,
        ):
            x_tiles = {}
            for tt in range(2):
                x_t = p1w.tile([128, C], F32, tag="x", name=f"x_t{tt}")
                nc.sync.dma_start(out=x_t[:], in_=xs_v[tt])
                x_tiles[tt] = x_t
            wkv_sb = wkvp.tile([128, 8, 2 * C], F32R)
            wkv_v = wkv.ap().rearrange("(cc p) o -> p cc o", p=128).bitcast(F32R)
            for oc in range(4):
                nc.sync.dma_start(out=wkv_sb[:, :, oc * 512:(oc + 1) * 512],
                                  in_=wkv_v[:, :, oc * 512:(oc + 1) * 512])
            kv_ps = [kvaccp.tile([128, 4, 65], F32, name=f"kv_ps{i}") for i in range(2)]

            for tt in range(NT):
                if tt in x_tiles:
                    x_t = x_tiles.pop(tt)
                else:
                    x_t = p1w.tile([128, C], F32, tag="x", name=f"x_t{tt}")
                    nc.sync.dma_start(out=x_t[:], in_=xs_v[tt])
                _emit_ln(nc, pools, x_t, eps_ln_t, x_t)
                # transpose h (=x_t) -> hT_full[:, :, tt*128:+128]
                tr_ps = trpsp.tile([128, 8, 128], F32)
                for cc in range(8):
                    nc.tensor.transpose(tr_ps[:, cc, :], x_t[:, cc * 128:(cc + 1) * 128], ident[:])
                tr_tmp = trtmpp.tile([128, 8, 128], F32)
                nc.vector.tensor_copy(out=tr_tmp[:], in_=tr_ps[:])
                nc.sync.dma_start(out=ht_d[tt], in_=tr_tmp[:])
                hT_r = htrp.tile([128, 8, 128], F32R)
                nc.sync.dma_start(out=hT_r[:], in_=tr_tmp[:].bitcast(F32R))
                # k, v for this tile
                k_sb = p1w.tile([128, C], F32, tag="k")
                v_ext = p1w.tile([128, H, 65], F32, tag="v")
                nc.vector.memset(v_ext[:, :, 64:65], 1.0)
                for oc in range(4):
                    ps = genpsp.tile([128, 512], F32, tag="gen")
                    for cc in range(8):
                        nc.tensor.matmul(ps[:], lhsT=hT_r[:, cc, :],
                                         rhs=wkv_sb[:, cc, oc * 512:(oc + 1) * 512],
                                         start=(cc == 0), stop=(cc == 7))
                    if oc < 2:  # k: phi = exp(min(x,0)) + relu(x)
                        ksl = k_sb[:, oc * 512:(oc + 1) * 512]
                        if has_bk:
                            nc.vector.tensor_tensor(out=ksl, in0=ps[:],
                                                    in1=bk_bc[:, oc * 512:(oc + 1) * 512], op=ALU.add)
                            src = ksl
                        else:
                            src = ps[:]
                        mt = p1w.tile([128, 512], F32, tag="phim")
                        nc.vector.tensor_scalar_min(out=mt[:], in0=src, scalar1=0.0)
                        nc.scalar.activation(out=mt[:], in_=mt[:], func=AF.Exp)
                        nc.vector.scalar_tensor_tensor(out=ksl, in0=src, scalar=0.0,
                                                       in1=mt[:], op0=ALU.max, op1=ALU.add)
                    else:      # v -> v_ext[:, heads, 0:64]
                        h0 = (oc - 2) * 8
                        dst = v_ext[:, h0:h0 + 8, 0:64]
                        if has_bv:
                            vb = bass.AP(tensor=bv.ap().tensor, offset=(oc - 2) * 512,
                                         ap=[[0, 128], [64, 8], [1, 64]])
                            vb_t = p1w.tile([128, 8, 64], F32, tag="vb")
                            nc.sync.dma_start(out=vb_t[:], in_=vb)
                            nc.vector.tensor_tensor(
                                out=dst, in0=ps[:].rearrange("p (h d) -> p h d", d=64),
                                in1=vb_t[:], op=ALU.add)
                        else:
                            nc.vector.tensor_copy(
                                out=dst, in_=ps[:].rearrange("p (h d) -> p h d", d=64))
                # kv accumulation: per head [64, 65] += k_h^T @ [v_h | 1]
                for h in range(H):
                    ti, hf, slot = h // 8, (h % 8) // 4, h % 4
                    nc.tensor.matmul(
                        kv_ps[ti][hf * 64:(hf + 1) * 64, slot, :],
                        lhsT=k_sb[:, h * 64:(h + 1) * 64],
                        rhs=v_ext[:, h, :],
                        start=(tt == 0), stop=(tt == NT - 1))

            # stage kv psum -> SBUF -> DRAM, then pairwise AllReduce
            kv_st = kvstagep.tile([128, 2, 4, 65], F32)
            for ti in range(2):
                nc.vector.tensor_copy(out=kv_st[:, ti], in_=kv_ps[ti][:])
                nc.sync.dma_start(out=cci[ti], in_=kv_st[:, ti])
            if no_cc:
                nc.sync.dma_start(out=cco[:], in_=cci[:])
            else:
                nc.gpsimd.collective_compute(
                    "AllReduce", ALU.add,
                    replica_groups=[[0, 1], [2, 3], [4, 5], [6, 7]],
                    ins=[cci[:]], outs=[cco[:]])

        # ------------- Phase 1b: qT (overlaps the collective) -------------
        big_cm = tc.tile_pool(name="big", bufs=4, side="right")
        big = big_cm.__enter__()
        qT_g = [big.tile([128, 8, 512], F32R, tag="grp", name=f"qT_g{i}") for i in range(NG)]
        with tc.tile_pool(name="wqp", bufs=1) as wqp, \
             tc.tile_pool(name="p1bw", bufs=3) as p1bw, \
             tc.tile_pool(name="qhtp", bufs=2) as qhtp, \
             tc.tile_pool(name="q_ps", bufs=4, space="PSUM") as qpsp:
            wq_sb = wqp.tile([128, 8, C], F32R)
            wq_v = wq.ap().rearrange("(cc p) o -> p cc o", p=128).bitcast(F32R)
            for oc in range(4):
                nc.sync.dma_start(out=wq_sb[:, :, oc * 256:(oc + 1) * 256],
                                  in_=wq_v[:, :, oc * 256:(oc + 1) * 256])
            for g in range(NG):
                qht = qhtp.tile([128, 8, 4, 128], F32R)
                nc.sync.dma_start(out=qht[:], in_=ht_d[4 * g:4 * (g + 1)].rearrange(
                    "tl p cc t -> p cc tl t").bitcast(F32R))
                qht_v = qht[:].rearrange("p cc tl t -> p cc (tl t)")
                for oc in range(8):
                    ps = qpsp.tile([128, 512], F32)
                    for cc in range(8):
                        nc.tensor.matmul(ps[:], lhsT=wq_sb[:, cc, oc * 128:(oc + 1) * 128],
                                         rhs=qht_v[:, cc, :],
                                         start=(cc == 0), stop=(cc == 7))
                    mt = p1bw.tile([128, 512], F32, tag="phim")
                    rt = p1bw.tile([128, 512], F32, tag="phir")
                    nc.vector.tensor_scalar(out=mt[:], in0=ps[:], scalar1=bq_sb[:, oc:oc + 1],
                                            scalar2=0.0, op0=ALU.add, op1=ALU.min)
                    nc.scalar.activation(out=mt[:], in_=mt[:], func=AF.Exp)
                    nc.vector.tensor_scalar(out=rt[:], in0=ps[:], scalar1=bq_sb[:, oc:oc + 1],
                                            scalar2=0.0, op0=ALU.add, op1=ALU.max)
                    nc.vector.tensor_tensor(out=mt[:], in0=mt[:], in1=rt[:], op=ALU.add)
                    nc.sync.dma_start(out=qT_g[g][:, oc, :], in_=mt[:].bitcast(F32R))

        # ---------------- Phase 2: attention + proj + LN2 ----------------
        with (
            tc.tile_pool(name="wpp", bufs=1) as wpp,
            tc.tile_pool(name="kv2", bufs=1) as kv2p,
            tc.tile_pool(name="p2w", bufs=2) as p2w,
            tc.tile_pool(name="p2w1", bufs=3) as p2w1,
            tc.tile_pool(name="attnt", bufs=1) as attntp,
            tc.tile_pool(name="zbcpa", bufs=1) as zbcpa,
            tc.tile_pool(name="z_ps", bufs=2, space="PSUM") as zpsp,
            tc.tile_pool(name="attn_ps", bufs=2, space="PSUM") as attnpsp,
            tc.tile_pool(name="proj_ps", bufs=2, space="PSUM") as projpsp,
            tc.tile_pool(name="tr2_ps", bufs=1, space="PSUM") as trps2p,
        ):
            wp_sb = wpp.tile([128, 8, C], F32R)
            nc.sync.dma_start(out=wp_sb[:], in_=wp.ap().rearrange(
                "(cc p) o -> p cc o", p=128).bitcast(F32R))
            kv_sb2 = kv2p.tile([128, 8, 65], F32R)
            kv_bd = kv2p.tile([128, 8, 128], F32R)
            nc.vector.memset(kv_bd[:].bitcast(F32), 0.0)
            bd = kv2p.tile([128, 8, 16], F32R)
            nc.vector.memset(bd[:].bitcast(F32), 0.0)
            for h in range(H):
                ti, hf, slot = h // 8, (h % 8) // 4, h % 4
                pbase = (h % 2) * 64
                nc.sync.dma_start(
                    out=kv_sb2[pbase:pbase + 64, h // 2, :],
                    in_=cco[ti, hf * 64:(hf + 1) * 64, slot, :].bitcast(F32R))
                nc.sync.dma_start(
                    out=kv_bd[pbase:pbase + 64, h // 2, pbase:pbase + 64],
                    in_=kv_sb2[pbase:pbase + 64, h // 2, 0:64])
                nc.sync.dma_start(
                    out=bd[pbase:pbase + 64, h // 2, h:h + 1],
                    in_=kv_sb2[pbase:pbase + 64, h // 2, 64:65])
            z_bcs = {}

            def emit_z(g):
                # z = 1 / (q . ksum + eps), then broadcast to head-pair layout
                zps = zpsp.tile([16, 512], F32, name=f"zps{g}", tag="zps")
                for pc in range(8):
                    nc.tensor.matmul(zps[:], lhsT=bd[:, pc, :], rhs=qT_g[g][:, pc, :],
                                     start=(pc == 0), stop=(pc == 7))
                zsl = p2w.tile([16, 512], F32, name=f"zt{g}", tag="zt")
                nc.vector.tensor_scalar_add(out=zsl[:], in0=zps[:], scalar1=EPS_ATTN)
                nc.vector.reciprocal(out=zsl[:], in_=zsl[:])
                nc.sync.dma_start(out=z_d[:, g * 512:(g + 1) * 512], in_=zsl[:])
                z_bc = zbcpa.tile([128, 8, 512], F32, name=f"zbc{g}", tag="zbc")
                zd_ap = z_d[:]
                for sub in range(2):
                    nc.sync.dma_start(
                        out=z_bc[sub * 64:(sub + 1) * 64, :, :],
                        in_=bass.AP(tensor=zd_ap.tensor,
                                    offset=zd_ap.offset + sub * TOK + g * 512,
                                    ap=[[0, 64], [2 * TOK, 8], [1, 512]]))
                z_bcs[g] = z_bc

            emit_z(0)
            for g in range(NG):
                if g + 1 < NG:
                    emit_z(g + 1)
                z_bc = z_bcs.pop(g)
                # attn_T = (kv_h^T q_h) * z, head pairs share a psum bank
                attn_r = attntp.tile([128, 8, 512], F32R)
                for cc in range(8):
                    aps = attnpsp.tile([128, 512], F32)
                    nc.tensor.matmul(aps[:], lhsT=kv_bd[:, cc, :],
                                     rhs=qT_g[g][:, cc, :], start=True, stop=True)
                    attn_tmp = p2w.tile([128, 512], F32, tag="attn_tmp", name=f"attn_tmp{g}_{cc}")
                    nc.vector.tensor_tensor(out=attn_tmp[:], in0=aps[:],
                                            in1=z_bc[:, cc, :], op=ALU.mult)
                    nc.sync.dma_start(out=attn_r[:, cc, :], in_=attn_tmp[:].bitcast(F32R))

                # proj + residual -> x1; LN2; transpose -> h2T group tile
                h2T = big.tile([128, 8, 512], F32R, tag="grp", name=f"h2T_g{g}")
                for tl in range(4):
                    tt = g * 4 + tl
                    x_rel = p2w.tile([128, C], F32, tag="xrel")
                    nc.sync.dma_start(out=x_rel[:], in_=xs_v[tt])
                    x1_t = p2w.tile([128, C], F32, tag="x1")
                    for oc in range(2):
                        pps = projpsp.tile([128, 512], F32)
                        for cc in range(8):
                            nc.tensor.matmul(pps[:], lhsT=attn_r[:, cc, tl * 128:(tl + 1) * 128],
                                             rhs=wp_sb[:, cc, oc * 512:(oc + 1) * 512],
                                             start=(cc == 0), stop=(cc == 7))
                        osl = slice(oc * 512, (oc + 1) * 512)
                        nc.vector.tensor_tensor(out=x1_t[:, osl], in0=pps[:],
                                                in1=x_rel[:, osl], op=ALU.add)
                        if has_bproj:
                            nc.vector.tensor_tensor(out=x1_t[:, osl], in0=x1_t[:, osl],
                                                    in1=bp_bc[:, osl], op=ALU.add)
                    nc.sync.dma_start(out=x1s[tt], in_=x1_t[:])
                    h2_t = p2w1.tile([128, C], F32, tag="h2")
                    _emit_ln(nc, pools, x1_t, eps_ln_t, h2_t)
                    tr_ps2 = trps2p.tile([128, 8, 128], F32)
                    for cc in range(8):
                        nc.tensor.transpose(tr_ps2[:, cc, :], h2_t[:, cc * 128:(cc + 1) * 128], ident[:])
                    tr_tmp2 = p2w1.tile([128, 8, 128], F32, tag="tr2")
                    nc.vector.tensor_copy(out=tr_tmp2[:], in_=tr_ps2[:])
                    nc.sync.dma_start(out=h2T[:, :, tl * 128:(tl + 1) * 128],
                                        in_=tr_tmp2[:].bitcast(F32R))
                qT_g[g] = h2T  # slot reuse: qT_g[g] fully consumed above

        h2T_g = qT_g  # now holds h2T group tiles

        w2_v = w2.ap().rearrange("(hc p) o -> p hc o", p=128).bitcast(F32R)

        # ---------------- Phase 3a: fc1 + gelu -> h3s (DRAM) ----------------
        with tc.tile_pool(name="w1p", bufs=2) as w1p, \
             tc.tile_pool(name="gelt", bufs=2) as geltp, \
             tc.tile_pool(name="f1_ps", bufs=4, space="PSUM") as f1psp:
            w1_v = w1.ap().rearrange("(cc p) o -> p cc o", p=128)
            for hd in range(32):
                w1_col = w1p.tile([128, 8, 128], F32R)
                nc.sync.dma_start(out=w1_col[:],
                                    in_=w1_v[:, :, hd * 128:(hd + 1) * 128].bitcast(F32R))
                for g in range(NG):
                    ps = f1psp.tile([128, 512], F32)
                    for cc in range(8):
                        nc.tensor.matmul(ps[:], lhsT=w1_col[:, cc, :],
                                         rhs=h2T_g[g][:, cc, :],
                                         start=(cc == 0), stop=(cc == 7))
                    gt = geltp.tile([128, 512], F32)
                    nc.scalar.activation(out=gt[:], in_=ps[:], func=AF.Gelu,
                                         bias=bg_sb[:, hd:hd + 1], scale=1.0)
                    nc.sync.dma_start(out=h3s[hd, :, g * 512:(g + 1) * 512], in_=gt[:])

        big_cm.__exit__(None, None, None)

        # ---------------- Phase 3b: fc2 + residual -> out ----------------
        with tc.tile_pool(name="w2p", bufs=1) as w2p, \
             tc.tile_pool(name="h3c", bufs=3) as h3cp, \
             tc.tile_pool(name="outp", bufs=2) as outp, \
             tc.tile_pool(name="f2_ps", bufs=3, space="PSUM") as f2psp:
            w2_sb = w2p.tile([128, 32, C], F32R)
            h3s_v = h3s[:].rearrange("hd p t -> p hd t")
            h3c_pre = {}
            nc.sync.dma_start(out=w2_sb[:, 0:4, :], in_=w2_v[:, 0:4, :])
            for tt in range(2):
                h3c = h3cp.tile([128, 32, 128], F32R, name=f"h3c{tt}", tag="h3c")
                nc.sync.dma_start(out=h3c[:],
                                  in_=h3s_v[:, :, tt * 128:(tt + 1) * 128].bitcast(F32R))
                h3c_pre[tt] = h3c
            for hc in range(1, 8):
                nc.sync.dma_start(out=w2_sb[:, 4 * hc:4 * (hc + 1), :],
                                  in_=w2_v[:, 4 * hc:4 * (hc + 1), :])
            for tt in range(NT):
                ps = f2psp.tile([128, C], F32)
                if tt in h3c_pre:
                    h3c = h3c_pre.pop(tt)
                else:
                    h3c = h3cp.tile([128, 32, 128], F32R, name=f"h3c{tt}", tag="h3c")
                    nc.sync.dma_start(out=h3c[:],
                                      in_=h3s_v[:, :, tt * 128:(tt + 1) * 128].bitcast(F32R))
                for hd in range(32):
                    for oc in range(2):
                        nc.tensor.matmul(ps[:, oc * 512:(oc + 1) * 512], lhsT=h3c[:, hd, :],
                                         rhs=w2_sb[:, hd, oc * 512:(oc + 1) * 512],
                                         start=(hd == 0), stop=(hd == 31))
                x1_rel = outp.tile([128, C], F32, tag="x1rel")
                nc.sync.dma_start(out=x1_rel[:], in_=x1s[tt])
                o_t = outp.tile([128, C], F32, tag="ot")
                nc.vector.tensor_tensor(out=o_t[:], in0=ps[:], in1=x1_rel[:], op=ALU.add)
                if has_bfc2:
                    nc.vector.tensor_tensor(out=o_t[:], in0=o_t[:], in1=b2_bc[:], op=ALU.add)
                nc.sync.dma_start(out=out_v[tt], in_=o_t[:])

    nc.compile()
    return nc


def _prep_inputs(x, norm1_g, norm1_b, qkv_w, proj_w, proj_b, norm2_g, norm2_b,
                 fc1_w, fc1_b, fc2_w, fc2_b):
    """Host-side weight prep. Folds LN gains into weights; LN biases into
    per-output biases. Returns (flags, per-core in_maps)."""
    x = np.asarray(x, np.float32)
    g1 = np.asarray(norm1_g, np.float32)
    b1 = np.asarray(norm1_b, np.float32)
    qkv_w = np.asarray(qkv_w, np.float32)
    proj_w = np.asarray(proj_w, np.float32)
    proj_b = np.asarray(proj_b, np.float32)
    g2 = np.asarray(norm2_g, np.float32)
    b2 = np.asarray(norm2_b, np.float32)
    fc1_w = np.asarray(fc1_w, np.float32)
    fc1_b = np.asarray(fc1_b, np.float32)
    fc2_w = np.asarray(fc2_w, np.float32)
    fc2_b = np.asarray(fc2_b, np.float32)

    wq_t = np.ascontiguousarray((qkv_w[0:C] * g1[None, :]).T)            # [c, o]
    wkv_t = np.ascontiguousarray((qkv_w[C:3 * C] * g1[None, :]).T)       # [c, 2C]
    wp_t = np.ascontiguousarray(proj_w.T)
    w1_t = np.ascontiguousarray((fc1_w * g2[None, :]).T)                 # [c, HID]
    w2_t = np.ascontiguousarray(fc2_w.T)                                 # [HID, c]
    bq_v = qkv_w[0:C] @ b1
    bk_v = qkv_w[C:2 * C] @ b1
    bv_v = qkv_w[2 * C:3 * C] @ b1
    bg_v = fc1_w @ b2 + fc1_b

    flags = (bool(np.any(bk_v)), bool(np.any(bv_v)),
             bool(np.any(proj_b)), bool(np.any(fc2_b)))

    shared = dict(wq=wq_t, wkv=wkv_t, wp=wp_t, w1=w1_t, w2=w2_t,
                  bq=np.ascontiguousarray(bq_v, dtype=np.float32),
                  bk=np.ascontiguousarray(bk_v, dtype=np.float32),
                  bv=np.ascontiguousarray(bv_v, dtype=np.float32),
                  bg=np.ascontiguousarray(bg_v, dtype=np.float32),
                  bp=proj_b, b2o=fc2_b)
    in_maps = []
    for core in range(8):
        b, half = core // 2, core % 2
        xs = np.ascontiguousarray(x[b, half * TOK:(half + 1) * TOK, :])
        in_maps.append({"xs": xs, **shared})
    return flags, in_maps


def get_compiled(flags):
    if flags not in _BUILD_CACHE:
        _BUILD_CACHE[flags] = _build(flags)
    return _BUILD_CACHE[flags]


def kernel(**inputs) -> np.ndarray:
    flags, in_maps = _prep_inputs(**inputs)
    nc = get_compiled(flags)
    res = run_bass_kernel_spmd(nc, in_maps=in_maps, core_ids=list(range(8)))
    shards = [res.results[c]["out"] for c in range(8)]
    full = np.empty((B, N, C), np.float32)
    for core in range(8):
        b, half = core // 2, core % 2
        full[b, half * TOK:(half + 1) * TOK, :] = shards[core]
    return full


# revision 32
# speedup vs baseline: 1.2003x; 1.0242x over previous
"""Trainium2 Bass kernel for nn_Block_9457517985872 (dense transformer block
with linear attention). Token-sharded across 8 NeuronCores: core c handles
batch c//2, sequence half c%2 (2048 tokens). Only cross-core communication is
a pairwise AllReduce of the per-head (kv, ksum) statistics [16,64,65] f32.

Self-contained: hardcodes all shapes from the problem spec.
"""
import numpy as np
from contextlib import ExitStack

import concourse.bass as bass
import concourse.tile as tile
from concourse import bacc, mybir
from concourse.bass_utils import run_bass_kernel_spmd
from concourse.masks import make_identity

F32 = mybir.dt.float32
F32R = mybir.dt.float32r
AF = mybir.ActivationFunctionType
ALU = mybir.AluOpType

B, N, C = 4, 4096, 1024
H, D = 16, 64
HID = 4096
TOK = 2048          # tokens per core
NT = TOK // 128     # 16 token tiles
NG = TOK // 512     # 4 token groups
EPS_LN = 1e-5
EPS_ATTN = 1e-6

_BUILD_CACHE = {}


def _emit_ln(nc, pools, x_t, eps_t, out_t):
    """LayerNorm core (no gamma/beta): out = (x - mean(x)) * rsqrt(var + eps).
    x_t: [128, 1024] f32 SBUF tile. out_t may alias x_t."""
    stats = pools["stat"].tile([128, 2, 6], F32, name="ln_stats", tag="ln_stats")
    mv = pools["stat"].tile([128, 2], F32, name="ln_mv", tag="ln_mv")
    for sg in range(2):
        nc.vector.bn_stats(out=stats[:, sg, :], in_=x_t[:, sg * 512:(sg + 1) * 512])
    nc.vector.bn_aggr(out=mv[:], in_=stats[:])
    # mv[:,0]=mean, mv[:,1]=var -> rstd
    nc.scalar.activation(out=mv[:, 1:2], in_=mv[:, 1:2], func=AF.Sqrt, bias=eps_t[:], scale=1.0)
    nc.vector.reciprocal(out=mv[:, 1:2], in_=mv[:, 1:2])
    # mv[:,0] = -mean*rstd
    nc.vector.tensor_tensor(out=mv[:, 0:1], in0=mv[:, 0:1], in1=mv[:, 1:2], op=ALU.mult)
    nc.vector.tensor_scalar_mul(out=mv[:, 0:1], in0=mv[:, 0:1], scalar1=-1.0)
    nc.scalar.activation(out=out_t[:], in_=x_t[:], func=AF.Identity,
                         bias=mv[:, 0:1], scale=mv[:, 1:2])


def _build(flags, no_cc=False):
    """flags: (has_bk, has_bv, has_bproj, has_bfc2)"""
    has_bk, has_bv, has_bproj, has_bfc2 = flags
    nc = bacc.Bacc("TRN2", target_bir_lowering=False, debug=False,
                   num_devices=1 if no_cc else 8)

    xs = nc.dram_tensor("xs", [TOK, C], F32, kind="ExternalInput")
    wq = nc.dram_tensor("wq", [C, C], F32, kind="ExternalInput")      # [c, o]
    wkv = nc.dram_tensor("wkv", [C, 2 * C], F32, kind="ExternalInput")
    wp = nc.dram_tensor("wp", [C, C], F32, kind="ExternalInput")
    w1 = nc.dram_tensor("w1", [C, HID], F32, kind="ExternalInput")
    w2 = nc.dram_tensor("w2", [HID, C], F32, kind="ExternalInput")
    bq = nc.dram_tensor("bq", [C], F32, kind="ExternalInput")
    bk = nc.dram_tensor("bk", [C], F32, kind="ExternalInput")
    bv = nc.dram_tensor("bv", [C], F32, kind="ExternalInput")
    bg = nc.dram_tensor("bg", [HID], F32, kind="ExternalInput")
    bp = nc.dram_tensor("bp", [C], F32, kind="ExternalInput")
    b2o = nc.dram_tensor("b2o", [C], F32, kind="ExternalInput")
    out = nc.dram_tensor("out", [TOK, C], F32, kind="ExternalOutput")

    xs_v = xs.ap().rearrange("(t p) c -> t p c", p=128)     # [16,128,1024]
    out_v = out.ap().rearrange("(t p) c -> t p c", p=128)

    with tile.TileContext(nc) as tc, ExitStack() as ctx:
        const = ctx.enter_context(tc.tile_pool(name="const", bufs=1))
        dram = ctx.enter_context(tc.tile_pool(name="dram", bufs=1, space="DRAM"))
        statp = ctx.enter_context(tc.tile_pool(name="stat", bufs=4))
        pools = {"stat": statp}

        ident = const.tile([128, 128], F32)
        make_identity(nc, ident[:])
        eps_ln_t = const.tile([128, 1], F32)
        nc.vector.memset(eps_ln_t[:], EPS_LN)
        bq_sb = const.tile([128, 8], F32)
        nc.sync.dma_start(out=bq_sb[:], in_=bq.ap().rearrange("(oc p) -> p oc", p=128))
        bg_sb = const.tile([128, 32], F32)
        nc.sync.dma_start(out=bg_sb[:], in_=bg.ap().rearrange("(hd p) -> p hd", p=128))
        if has_bk:
            bk_bc = const.tile([128, C], F32)
            nc.sync.dma_start(out=bk_bc[:], in_=bass.AP(
                tensor=bk.ap().tensor, offset=0, ap=[[0, 128], [1, C]]))
        if has_bproj:
            bp_bc = const.tile([128, C], F32)
            nc.sync.dma_start(out=bp_bc[:], in_=bass.AP(
                tensor=bp.ap().tensor, offset=0, ap=[[0, 128], [1, C]]))
        if has_bfc2:
            b2_bc = const.tile([128, C], F32)
            nc.sync.dma_start(out=b2_bc[:], in_=bass.AP(
                tensor=b2o.ap().tensor, offset=0, ap=[[0, 128], [1, C]]))

        x1s = dram.tile([NT, 128, C], F32)
        h3s = dram.tile([32, 128, TOK], F32)
        cci = dram.tile([2, 128, 4, 65], F32)
        cco = dram.tile([2, 128, 4, 65], F32)
        z_d = dram.tile([16, TOK], F32)
        ht_d = dram.tile([NT, 128, 8, 128], F32)

        # ---------------- Phase 1a: LN1, hT, k/v, kv+ksum ----------------
        with (
            tc.tile_pool(name="wkvp", bufs=1) as wkvp,
            tc.tile_pool(name="p1w", bufs=3) as p1w,
            tc.tile_pool(name="htrp", bufs=2) as htrp,
            tc.tile_pool(name="trtmp", bufs=2) as trtmpp,
            tc.tile_pool(name="kvstage", bufs=1) as kvstagep,
            tc.tile_pool(name="kvacc_ps", bufs=1, space="PSUM") as kvaccp,
            tc.tile_pool(name="tr_ps", bufs=1, space="PSUM") as trpsp,
            tc.tile_pool(name="gen_ps", bufs=4, space="PSUM") as genpsp,
        ):
            x_tiles = {}
            for tt in range(2):
                x_t = p1w.tile([128, C], F32, tag="x", name=f"x_t{tt}")
                nc.sync.dma_start(out=x_t[:], in_=xs_v[tt])
                x_tiles[tt] = x_t
            wkv_sb = wkvp.tile([128, 8, 2 * C], F32R)
            wkv_v = wkv.ap().rearrange("(cc p) o -> p cc o", p=128).bitcast(F32R)
            for oc in range(4):
                nc.sync.dma_start(out=wkv_sb[:, :, oc * 512:(oc + 1) * 512],
                                  in_=wkv_v[:, :, oc * 512:(oc + 1) * 512])
            kv_ps = [kvaccp.tile([128, 4, 65], F32, name=f"kv_ps{i}") for i in range(2)]

            for tt in range(NT):
                if tt in x_tiles:
                    x_t = x_tiles.pop(tt)
                else:
                    x_t = p1w.tile([128, C], F32, tag="x", name=f"x_t{tt}")
                    nc.sync.dma_start(out=x_t[:], in_=xs_v[tt])
                _emit_ln(nc, pools, x_t, eps_ln_t, x_t)
                # transpose h (=x_t) -> hT_full[:, :, tt*128:+128]
                tr_ps = trpsp.tile([128, 8, 128], F32)
                for cc in range(8):
                    nc.tensor.transpose(tr_ps[:, cc, :], x_t[:, cc * 128:(cc + 1) * 128], ident[:])
                tr_tmp = trtmpp.tile([128, 8, 128], F32)
                nc.vector.tensor_copy(out=tr_tmp[:], in_=tr_ps[:])
                nc.sync.dma_start(out=ht_d[tt], in_=tr_tmp[:])
                hT_r = htrp.tile([128, 8, 128], F32R)
                nc.sync.dma_start(out=hT_r[:], in_=tr_tmp[:].bitcast(F32R))
                # k, v for this tile
                k_sb = p1w.tile([128, C], F32, tag="k")
                v_ext = p1w.tile([128, H, 65], F32, tag="v")
                nc.vector.memset(v_ext[:, :, 64:65], 1.0)
                for oc in range(4):
                    ps = genpsp.tile([128, 512], F32, tag="gen")
                    for cc in range(8):
                        nc.tensor.matmul(ps[:], lhsT=hT_r[:, cc, :],
                                         rhs=wkv_sb[:, cc, oc * 512:(oc + 1) * 512],
                                         start=(cc == 0), stop=(cc == 7))
                    if oc < 2:  # k: phi = exp(min(x,0)) + relu(x)
                        ksl = k_sb[:, oc * 512:(oc + 1) * 512]
                        if has_bk:
                            nc.vector.tensor_tensor(out=ksl, in0=ps[:],
                                                    in1=bk_bc[:, oc * 512:(oc + 1) * 512], op=ALU.add)
                            src = ksl
                        else:
                            src = ps[:]
                        mt = p1w.tile([128, 512], F32, tag="phim")
                        nc.vector.tensor_scalar_min(out=mt[:], in0=src, scalar1=0.0)
                        nc.scalar.activation(out=mt[:], in_=mt[:], func=AF.Exp)
                        nc.vector.scalar_tensor_tensor(out=ksl, in0=src, scalar=0.0,
                                                       in1=mt[:], op0=ALU.max, op1=ALU.add)
                    else:      # v -> v_ext[:, heads, 0:64]
                        h0 = (oc - 2) * 8
                        dst = v_ext[:, h0:h0 + 8, 0:64]
                        if has_bv:
                            vb = bass.AP(tensor=bv.ap().tensor, offset=(oc - 2) * 512,
                                         ap=[[0, 128], [64, 8], [1, 64]])
                            vb_t = p1w.tile([128, 8, 64], F32, tag="vb")
                            nc.sync.dma_start(out=vb_t[:], in_=vb)
                            nc.vector.tensor_tensor(
                                out=dst, in0=ps[:].rearrange("p (h d) -> p h d", d=64),
                                in1=vb_t[:], op=ALU.add)
                        else:
                            nc.vector.tensor_copy(
                                out=dst, in_=ps[:].rearrange("p (h d) -> p h d", d=64))
                # kv accumulation: per head [64, 65] += k_h^T @ [v_h | 1]
                for h in range(H):
                    ti, hf, slot = h // 8, (h % 8) // 4, h % 4
                    nc.tensor.matmul(
                        kv_ps[ti][hf * 64:(hf + 1) * 64, slot, :],
                        lhsT=k_sb[:, h * 64:(h + 1) * 64],
                        rhs=v_ext[:, h, :],
                        start=(tt == 0), stop=(tt == NT - 1))

            # stage kv psum -> SBUF -> DRAM, then pairwise AllReduce
            kv_st = kvstagep.tile([128, 2, 4, 65], F32)
            for ti in range(2):
                nc.vector.tensor_copy(out=kv_st[:, ti], in_=kv_ps[ti][:])
                nc.sync.dma_start(out=cci[ti], in_=kv_st[:, ti])
            if no_cc:
                nc.sync.dma_start(out=cco[:], in_=cci[:])
            else:
                nc.gpsimd.collective_compute(
                    "AllReduce", ALU.add,
                    replica_groups=[[0, 1], [2, 3], [4, 5], [6, 7]],
                    ins=[cci[:]], outs=[cco[:]])

        # ------------- Phase 1b: qT (overlaps the collective) -------------
        big_cm = tc.tile_pool(name="big", bufs=4, side="right")
        big = big_cm.__enter__()
        qT_g = [big.tile([128, 8, 512], F32R, tag="grp", name=f"qT_g{i}") for i in range(NG)]
        with tc.tile_pool(name="wqp", bufs=1) as wqp, \
             tc.tile_pool(name="p1bw", bufs=3) as p1bw, \
             tc.tile_pool(name="qhtp", bufs=3) as qhtp, \
             tc.tile_pool(name="q_ps", bufs=4, space="PSUM") as qpsp:
            wq_sb = wqp.tile([128, 8, C], F32R)
            wq_v = wq.ap().rearrange("(cc p) o -> p cc o", p=128).bitcast(F32R)
            for oc in range(4):
                nc.sync.dma_start(out=wq_sb[:, :, oc * 256:(oc + 1) * 256],
                                  in_=wq_v[:, :, oc * 256:(oc + 1) * 256])
            for g in range(NG):
                qht = qhtp.tile([128, 8, 4, 128], F32R)
                nc.sync.dma_start(out=qht[:], in_=ht_d[4 * g:4 * (g + 1)].rearrange(
                    "tl p cc t -> p cc tl t").bitcast(F32R))
                qht_v = qht[:].rearrange("p cc tl t -> p cc (tl t)")
                for oc in range(8):
                    ps = qpsp.tile([128, 512], F32)
                    for cc in range(8):
                        nc.tensor.matmul(ps[:], lhsT=wq_sb[:, cc, oc * 128:(oc + 1) * 128],
                                         rhs=qht_v[:, cc, :],
                                         start=(cc == 0), stop=(cc == 7))
                    mt = p1bw.tile([128, 512], F32, tag="phim")
                    rt = p1bw.tile([128, 512], F32, tag="phir")
                    nc.vector.tensor_scalar(out=mt[:], in0=ps[:], scalar1=bq_sb[:, oc:oc + 1],
                                            scalar2=0.0, op0=ALU.add, op1=ALU.min)
                    nc.scalar.activation(out=mt[:], in_=mt[:], func=AF.Exp)
                    nc.vector.tensor_scalar(out=rt[:], in0=ps[:], scalar1=bq_sb[:, oc:oc + 1],
                                            scalar2=0.0, op0=ALU.add, op1=ALU.max)
                    nc.vector.tensor_tensor(out=mt[:], in0=mt[:], in1=rt[:], op=ALU.add)
                    nc.sync.dma_start(out=qT_g[g][:, oc, :], in_=mt[:].bitcast(F32R))

        # ---------------- Phase 2: attention + proj + LN2 ----------------
        with (
            tc.tile_pool(name="wpp", bufs=1) as wpp,
            tc.tile_pool(name="kv2", bufs=1) as kv2p,
            tc.tile_pool(name="p2w", bufs=2) as p2w,
            tc.tile_pool(name="p2w1", bufs=3) as p2w1,
            tc.tile_pool(name="attnt", bufs=1) as attntp,
            tc.tile_pool(name="zbcpa", bufs=1) as zbcpa,
            tc.tile_pool(name="z_ps", bufs=1, space="PSUM") as zpsp,
            tc.tile_pool(name="attn_ps", bufs=3, space="PSUM") as attnpsp,
            tc.tile_pool(name="proj_ps", bufs=2, space="PSUM") as projpsp,
            tc.tile_pool(name="tr2_ps", bufs=1, space="PSUM") as trps2p,
        ):
            wp_sb = wpp.tile([128, 8, C], F32R)
            nc.sync.dma_start(out=wp_sb[:], in_=wp.ap().rearrange(
                "(cc p) o -> p cc o", p=128).bitcast(F32R))
            kv_sb2 = kv2p.tile([128, 8, 65], F32R)
            kv_bd = kv2p.tile([128, 8, 128], F32R)
            nc.vector.memset(kv_bd[:].bitcast(F32), 0.0)
            bd = kv2p.tile([128, 8, 16], F32R)
            nc.vector.memset(bd[:].bitcast(F32), 0.0)
            for h in range(H):
                ti, hf, slot = h // 8, (h % 8) // 4, h % 4
                pbase = (h % 2) * 64
                nc.sync.dma_start(
                    out=kv_sb2[pbase:pbase + 64, h // 2, :],
                    in_=cco[ti, hf * 64:(hf + 1) * 64, slot, :].bitcast(F32R))
                nc.sync.dma_start(
                    out=kv_bd[pbase:pbase + 64, h // 2, pbase:pbase + 64],
                    in_=kv_sb2[pbase:pbase + 64, h // 2, 0:64])
                nc.sync.dma_start(
                    out=bd[pbase:pbase + 64, h // 2, h:h + 1],
                    in_=kv_sb2[pbase:pbase + 64, h // 2, 64:65])
            z_bcs = {}

            def emit_z(g):
                # z = 1 / (q . ksum + eps), then broadcast to head-pair layout
                zps = zpsp.tile([16, 512], F32, name=f"zps{g}", tag="zps")
                for pc in range(8):
                    nc.tensor.matmul(zps[:], lhsT=bd[:, pc, :], rhs=qT_g[g][:, pc, :],
                                     start=(pc == 0), stop=(pc == 7))
                zsl = p2w.tile([16, 512], F32, name=f"zt{g}", tag="zt")
                nc.vector.tensor_scalar_add(out=zsl[:], in0=zps[:], scalar1=EPS_ATTN)
                nc.vector.reciprocal(out=zsl[:], in_=zsl[:])
                nc.sync.dma_start(out=z_d[:, g * 512:(g + 1) * 512], in_=zsl[:])
                z_bc = zbcpa.tile([128, 8, 512], F32, name=f"zbc{g}", tag="zbc")
                zd_ap = z_d[:]
                for sub in range(2):
                    nc.sync.dma_start(
                        out=z_bc[sub * 64:(sub + 1) * 64, :, :],
                        in_=bass.AP(tensor=zd_ap.tensor,
                                    offset=zd_ap.offset + sub * TOK + g * 512,
                                    ap=[[0, 64], [2 * TOK, 8], [1, 512]]))
                z_bcs[g] = z_bc

            emit_z(0)
            for g in range(NG):
                if g + 1 < NG:
                    emit_z(g + 1)
                z_bc = z_bcs.pop(g)
                # attn_T = (kv_h^T q_h) * z, head pairs share a psum bank
                attn_r = attntp.tile([128, 8, 512], F32R)
                for cc in range(8):
                    aps = attnpsp.tile([128, 512], F32)
                    nc.tensor.matmul(aps[:], lhsT=kv_bd[:, cc, :],
                                     rhs=qT_g[g][:, cc, :], start=True, stop=True)
                    attn_tmp = p2w.tile([128, 512], F32, tag="attn_tmp", name=f"attn_tmp{g}_{cc}")
                    nc.vector.tensor_tensor(out=attn_tmp[:], in0=aps[:],
                                            in1=z_bc[:, cc, :], op=ALU.mult)
                    nc.sync.dma_start(out=attn_r[:, cc, :], in_=attn_tmp[:].bitcast(F32R))

                # proj + residual -> x1; LN2; transpose -> h2T group tile
                h2T = big.tile([128, 8, 512], F32R, tag="grp", name=f"h2T_g{g}")
                for tl in range(4):
                    tt = g * 4 + tl
                    x_rel = p2w.tile([128, C], F32, tag="xrel")
                    nc.sync.dma_start(out=x_rel[:], in_=xs_v[tt])
                    x1_t = p2w.tile([128, C], F32, tag="x1")
                    for oc in range(2):
                        pps = projpsp.tile([128, 512], F32)
                        for cc in range(8):
                            nc.tensor.matmul(pps[:], lhsT=attn_r[:, cc, tl * 128:(tl + 1) * 128],
                                             rhs=wp_sb[:, cc, oc * 512:(oc + 1) * 512],
                                             start=(cc == 0), stop=(cc == 7))
                        osl = slice(oc * 512, (oc + 1) * 512)
                        nc.vector.tensor_tensor(out=x1_t[:, osl], in0=pps[:],
                                                in1=x_rel[:, osl], op=ALU.add)
                        if has_bproj:
                            nc.vector.tensor_tensor(out=x1_t[:, osl], in0=x1_t[:, osl],
                                                    in1=bp_bc[:, osl], op=ALU.add)
                    nc.sync.dma_start(out=x1s[tt], in_=x1_t[:])
                    h2_t = p2w1.tile([128, C], F32, tag="h2")
                    _emit_ln(nc, pools, x1_t, eps_ln_t, h2_t)
                    tr_ps2 = trps2p.tile([128, 8, 128], F32)
                    for cc in range(8):
                        nc.tensor.transpose(tr_ps2[:, cc, :], h2_t[:, cc * 128:(cc + 1) * 128], ident[:])
                    tr_tmp2 = p2w1.tile([128, 8, 128], F32, tag="tr2")
                    nc.vector.tensor_copy(out=tr_tmp2[:], in_=tr_ps2[:])
                    nc.sync.dma_start(out=h2T[:, :, tl * 128:(tl + 1) * 128],
                                        in_=tr_tmp2[:].bitcast(F32R))
                qT_g[g] = h2T  # slot reuse: qT_g[g] fully consumed above

        h2T_g = qT_g  # now holds h2T group tiles

        w2_v = w2.ap().rearrange("(hc p) o -> p hc o", p=128).bitcast(F32R)

        # ---------------- Phase 3a: fc1 + gelu -> h3s (DRAM) ----------------
        with tc.tile_pool(name="w1p", bufs=2) as w1p, \
             tc.tile_pool(name="gelt", bufs=2) as geltp, \
             tc.tile_pool(name="f1_ps", bufs=4, space="PSUM") as f1psp:
            w1_v = w1.ap().rearrange("(cc p) o -> p cc o", p=128)
            for hd in range(32):
                w1_col = w1p.tile([128, 8, 128], F32R)
                nc.sync.dma_start(out=w1_col[:],
                                    in_=w1_v[:, :, hd * 128:(hd + 1) * 128].bitcast(F32R))
                for g in range(NG):
                    ps = f1psp.tile([128, 512], F32)
                    for cc in range(8):
                        nc.tensor.matmul(ps[:], lhsT=w1_col[:, cc, :],
                                         rhs=h2T_g[g][:, cc, :],
                                         start=(cc == 0), stop=(cc == 7))
                    gt = geltp.tile([128, 512], F32)
                    nc.scalar.activation(out=gt[:], in_=ps[:], func=AF.Gelu,
                                         bias=bg_sb[:, hd:hd + 1], scale=1.0)
                    nc.sync.dma_start(out=h3s[hd, :, g * 512:(g + 1) * 512], in_=gt[:])

        big_cm.__exit__(None, None, None)

        # ---------------- Phase 3b: fc2 + residual -> out ----------------
        with tc.tile_pool(name="w2p", bufs=1) as w2p, \
             tc.tile_pool(name="h3c", bufs=3) as h3cp, \
             tc.tile_pool(name="outp", bufs=2) as outp, \
             tc.tile_pool(name="f2_ps", bufs=3, space="PSUM") as f2psp:
            w2_sb = w2p.tile([128, 32, C], F32R)
            h3s_v = h3s[:].rearrange("hd p t -> p hd t")
            h3c_pre = {}
            nc.sync.dma_start(out=w2_sb[:, 0:4, :], in_=w2_v[:, 0:4, :])
            for tt in range(3):
                h3c = h3cp.tile([128, 32, 128], F32R, name=f"h3c{tt}", tag="h3c")
                nc.sync.dma_start(out=h3c[:],
                                  in_=h3s_v[:, :, tt * 128:(tt + 1) * 128].bitcast(F32R))
                h3c_pre[tt] = h3c
            for hc in range(1, 8):
                nc.sync.dma_start(out=w2_sb[:, 4 * hc:4 * (hc + 1), :],
                                  in_=w2_v[:, 4 * hc:4 * (hc + 1), :])
            for tt in range(NT):
                ps = f2psp.tile([128, C], F32)
                if tt in h3c_pre:
                    h3c = h3c_pre.pop(tt)
                else:
                    h3c = h3cp.tile([128, 32, 128], F32R, name=f"h3c{tt}", tag="h3c")
                    nc.sync.dma_start(out=h3c[:],
                                      in_=h3s_v[:, :, tt * 128:(tt + 1) * 128].bitcast(F32R))
                for hd in range(32):
                    for oc in range(2):
                        nc.tensor.matmul(ps[:, oc * 512:(oc + 1) * 512], lhsT=h3c[:, hd, :],
                                         rhs=w2_sb[:, hd, oc * 512:(oc + 1) * 512],
                                         start=(hd == 0), stop=(hd == 31))
                x1_rel = outp.tile([128, C], F32, tag="x1rel")
                nc.sync.dma_start(out=x1_rel[:], in_=x1s[tt])
                o_t = outp.tile([128, C], F32, tag="ot")
                nc.vector.tensor_tensor(out=o_t[:], in0=ps[:], in1=x1_rel[:], op=ALU.add)
                if has_bfc2:
                    nc.vector.tensor_tensor(out=o_t[:], in0=o_t[:], in1=b2_bc[:], op=ALU.add)
                nc.sync.dma_start(out=out_v[tt], in_=o_t[:])

    nc.compile()
    return nc


def _prep_inputs(x, norm1_g, norm1_b, qkv_w, proj_w, proj_b, norm2_g, norm2_b,
                 fc1_w, fc1_b, fc2_w, fc2_b):
    """Host-side weight prep. Folds LN gains into weights; LN biases into
    per-output biases. Returns (flags, per-core in_maps)."""
    x = np.asarray(x, np.float32)
    g1 = np.asarray(norm1_g, np.float32)
    b1 = np.asarray(norm1_b, np.float32)
    qkv_w = np.asarray(qkv_w, np.float32)
    proj_w = np.asarray(proj_w, np.float32)
    proj_b = np.asarray(proj_b, np.float32)
    g2 = np.asarray(norm2_g, np.float32)
    b2 = np.asarray(norm2_b, np.float32)
    fc1_w = np.asarray(fc1_w, np.float32)
    fc1_b = np.asarray(fc1_b, np.float32)
    fc2_w = np.asarray(fc2_w, np.float32)
    fc2_b = np.asarray(fc2_b, np.float32)

    wq_t = np.ascontiguousarray((qkv_w[0:C] * g1[None, :]).T)            # [c, o]
    wkv_t = np.ascontiguousarray((qkv_w[C:3 * C] * g1[None, :]).T)       # [c, 2C]
    wp_t = np.ascontiguousarray(proj_w.T)
    w1_t = np.ascontiguousarray((fc1_w * g2[None, :]).T)                 # [c, HID]
    w2_t = np.ascontiguousarray(fc2_w.T)                                 # [HID, c]
    bq_v = qkv_w[0:C] @ b1
    bk_v = qkv_w[C:2 * C] @ b1
    bv_v = qkv_w[2 * C:3 * C] @ b1
    bg_v = fc1_w @ b2 + fc1_b

    flags = (bool(np.any(bk_v)), bool(np.any(bv_v)),
             bool(np.any(proj_b)), bool(np.any(fc2_b)))

    shared = dict(wq=wq_t, wkv=wkv_t, wp=wp_t, w1=w1_t, w2=w2_t,
                  bq=np.ascontiguousarray(bq_v, dtype=np.float32),
                  bk=np.ascontiguousarray(bk_v, dtype=np.float32),
                  bv=np.ascontiguousarray(bv_v, dtype=np.float32),
                  bg=np.ascontiguousarray(bg_v, dtype=np.float32),
                  bp=proj_b, b2o=fc2_b)
    in_maps = []
    for core in range(8):
        b, half = core // 2, core % 2
        xs = np.ascontiguousarray(x[b, half * TOK:(half + 1) * TOK, :])
        in_maps.append({"xs": xs, **shared})
    return flags, in_maps


def get_compiled(flags):
    if flags not in _BUILD_CACHE:
        _BUILD_CACHE[flags] = _build(flags)
    return _BUILD_CACHE[flags]


def kernel(**inputs) -> np.ndarray:
    flags, in_maps = _prep_inputs(**inputs)
    nc = get_compiled(flags)
    res = run_bass_kernel_spmd(nc, in_maps=in_maps, core_ids=list(range(8)))
    shards = [res.results[c]["out"] for c in range(8)]
    full = np.empty((B, N, C), np.float32)
    for core in range(8):
        b, half = core // 2, core % 2
        full[b, half * TOK:(half + 1) * TOK, :] = shards[core]
    return full
